# revision 1
# baseline (speedup 1.0000x reference)
"""InternLM3 self-attention (prefill, GQA, RoPE) on 8 Trainium2 cores.

Tensor-parallel over heads: core r owns q heads 4r..4r+3 and kv head r
(wqkv column shards, wo row shards).  Each core computes its partial
output projection; the 8 partials are summed on the host (an on-device
all-reduce of 32 MB runs at ~32 GB/s through ncfw and would dominate the
kernel, so the reduction is done host-side).

Matmuls run in float32r (TF32-like fast fp32 mode, 1 cycle/row at
N>=512 vs 4 for plain fp32) with fp32 PSUM accumulation.

Device-side layout trick: everything is computed transposed
(qkv^T = wqkv_shard^T @ hidden^T) so that
  - wqkv loads land directly as the stationary operand,
  - q^T/k^T slices feed the scores matmul with head_dim on partitions,
  - scores come out as S^T [k, q], so exp(S^T) feeds the PV matmul
    directly (contraction over k on partitions) with zero transposes,
  - attn^T slices are exactly the stationary operand of the wo matmul.
The only transposes are hidden^T (done host-side, it is an input-layout
choice) and v^T -> v (16 tiny PE transposes).
"""

import numpy as np

import concourse.bass as bass
import concourse.bacc as bacc
import concourse.mybir as mybir
import concourse.tile as tile
from concourse.bass_utils import run_bass_kernel_spmd

T = 2048
H = 4096
NH = 32
NKV = 8
HD = 128
HALF = HD // 2
BASE = 1000000.0
NCORES = 8
QH = NH // NCORES            # 4 q heads per core
QCOLS = QH * HD              # 512
SH_COLS = QCOLS + 2 * HD     # 768 wqkv cols per core
NEG = -1e30

P = 128
TC = 512                     # token chunk (matmul moving dim)
NT = T // TC                 # 4
NHC = H // P                 # 32 contraction chunks for qkv
NQC = SH_COLS // P           # 6 qkv col chunks
NKC = T // P                 # 16 k chunks
NOC = H // TC                # 8 output col chunks
NTC16 = T // P               # 16 token chunks of 128

f32 = mybir.dt.float32
f32r = mybir.dt.float32r

_COMPILED = None


def _build():
    nc = bacc.Bacc("TRN2", target_bir_lowering=False, debug=False,
                   num_devices=NCORES)

    hidT = nc.dram_tensor("hidT", [H, T], f32r, kind="ExternalInput").ap()
    wqkv_s = nc.dram_tensor("wqkv_s", [H, SH_COLS], f32r,
                            kind="ExternalInput").ap()
    wo_s = nc.dram_tensor("wo_s", [QCOLS, H], f32r,
                          kind="ExternalInput").ap()
    cosq = nc.dram_tensor("cosq", [P, T], f32, kind="ExternalInput").ap()
    sinq = nc.dram_tensor("sinq", [P, T], f32, kind="ExternalInput").ap()
    cosk = nc.dram_tensor("cosk", [P, T], f32, kind="ExternalInput").ap()
    sink = nc.dram_tensor("sink", [P, T], f32, kind="ExternalInput").ap()
    masks = nc.dram_tensor("masks", [P, 4, TC], f32,
                           kind="ExternalInput").ap()
    rperm = nc.dram_tensor("rperm", [P, P], f32r, kind="ExternalInput").ap()
    ident = nc.dram_tensor("ident", [P, P], f32r, kind="ExternalInput").ap()
    ones_k = nc.dram_tensor("ones_k", [P, 1], f32r,
                            kind="ExternalInput").ap()
    ones_m = nc.dram_tensor("ones_m", [1, P], f32r,
                            kind="ExternalInput").ap()
    part = nc.dram_tensor("part", [T, H], f32, kind="ExternalOutput").ap()

    with tile.TileContext(nc) as tc:
        with tc.tile_pool(name="keep", bufs=1) as keep:
            # long-lived SBUF: qkv^T [128, 6, 2048] f32r (48 KB/part)
            qkvT = keep.tile([P, NQC, T], f32r)

            # constants first: tiny DMAs, land before the bulk loads
            ct = keep.tile([P, T], f32, tag="cosq_t")
            st = keep.tile([P, T], f32, tag="sinq_t")
            ctk = keep.tile([P, T], f32, tag="cosk_t")
            stk = keep.tile([P, T], f32, tag="sink_t")
            mt = keep.tile([P, 4, TC], f32, tag="masks_t")
            rp = keep.tile([P, P], f32r, tag="rperm_t")
            idt = keep.tile([P, P], f32r, tag="ident_t")
            o_k = keep.tile([P, 1], f32r, tag="ones_k_t")
            o_m = keep.tile([1, P], f32r, tag="ones_m_t")

            # ---------------- phase 1: qkv^T = wqkv^T @ hidden^T -------
            with tc.tile_pool(name="wq", bufs=1) as wqp, \
                 tc.tile_pool(name="hstream", bufs=4) as hsp, \
                 tc.tile_pool(name="qps", bufs=1, space="PSUM") as qpsp:
                wq = wqp.tile([P, NHC, SH_COLS], f32r)
                for h in range(NHC):
                    nc.sync.dma_start(
                        wq[:, h, :], wqkv_s[h * P:(h + 1) * P, :])
                nc.sync.dma_start(ct[:], cosq[:])
                nc.sync.dma_start(st[:], sinq[:])
                nc.sync.dma_start(ctk[:], cosk[:])
                nc.sync.dma_start(stk[:], sink[:])
                nc.sync.dma_start(mt[:], masks[:])
                nc.sync.dma_start(rp[:], rperm[:])
                nc.sync.dma_start(idt[:], ident[:])
                nc.sync.dma_start(o_k[:], ones_k[:])
                nc.sync.dma_start(o_m[:], ones_m[:])
                for t in range(NT):
                    qps = [qpsp.tile([P, TC], f32, tag=f"qps{c}",
                                     name=f"qps{c}_{t}")
                           for c in range(NQC)]
                    for h in range(NHC):
                        ht = hsp.tile([P, TC], f32r, tag="ht")
                        nc.scalar.dma_start(
                            ht[:], hidT[h * P:(h + 1) * P,
                                        t * TC:(t + 1) * TC])
                        for c in range(NQC):
                            nc.tensor.matmul(
                                qps[c][:], wq[:, h, c * P:(c + 1) * P],
                                ht[:], start=(h == 0), stop=(h == NHC - 1))
                    for c in range(NQC):
                        nc.scalar.copy(
                            qkvT[:, c, t * TC:(t + 1) * TC], qps[c][:])

            with tc.tile_pool(name="keep2", bufs=1) as keep2:
                    # ---------------- phase 3: v_nat = v^T transposed ----------
                vnat = keep2.tile([P, NKC, P], f32r, tag="vnat")
                with tc.tile_pool(name="vt_ps", bufs=4, space="PSUM") as vps:
                    for kc in range(NKC):
                        tp = vps.tile([P, P], f32r, tag="vtp")
                        nc.tensor.transpose(
                            tp[:], qkvT[:, 5, kc * P:(kc + 1) * P], idt[:])
                        nc.scalar.copy(vnat[:, kc, :], tp[:])

                # ---------------- phase 2: RoPE on q (scaled) and k --------
                with tc.tile_pool(name="rope_sb", bufs=4) as rsb, \
                     tc.tile_pool(name="rope_ps", bufs=4, space="PSUM") as rps:
                    for idx in range(QH + 1):        # 4 q heads + 1 k head
                        cos_t, sin_t = (ct, st) if idx < QH else (ctk, stk)
                        for t in range(NT):
                            sl = slice(t * TC, (t + 1) * TC)
                            x = qkvT[:, idx, sl]
                            rot = rps.tile([P, TC], f32, tag="rot")
                            nc.tensor.matmul(rot[:], rp[:], x,
                                             start=True, stop=True)
                            tmp = rsb.tile([P, TC], f32, tag="rtmp")
                            nc.vector.tensor_tensor(
                                tmp[:], rot[:], sin_t[:, sl],
                                mybir.AluOpType.mult)
                            nc.vector.tensor_tensor(
                                x, x.bitcast(f32), cos_t[:, sl],
                                mybir.AluOpType.mult)
                            nc.vector.tensor_tensor(
                                x, x.bitcast(f32), tmp[:],
                                mybir.AluOpType.add)

                # ---------------- phase 4: causal attention ----------------
                attnT = keep2.tile([P, QH, T], f32r, tag="attnT")
                with tc.tile_pool(name="att_sb", bufs=8) as asb, \
                     tc.tile_pool(name="att_sm", bufs=4) as asm_p, \
                     tc.tile_pool(name="st_ps", bufs=3, space="PSUM") as stp, \
                     tc.tile_pool(name="pv_ps", bufs=2, space="PSUM") as pvp, \
                     tc.tile_pool(name="d_ps", bufs=2, space="PSUM") as dpp, \
                     tc.tile_pool(name="rb_ps", bufs=1, space="PSUM") as rbp:
                    for head in range(QH):
                        for g in range(NT):
                            kmax = (NT // 1) * (g + 1)   # 4*(g+1) k chunks
                            qsl = slice(g * TC, (g + 1) * TC)
                            d_ps = dpp.tile([1, TC], f32, tag="d")
                            pv = pvp.tile([P, TC], f32, tag="pv")
                            es = asb.tile([P, TC], f32r, tag="esum")
                            e_prev = None
                            for kc in range(kmax):
                                st_ps = stp.tile([P, TC], f32, tag="st")
                                nc.tensor.matmul(
                                    st_ps[:],
                                    qkvT[:, QH, kc * P:(kc + 1) * P],
                                    qkvT[:, head, qsl],
                                    start=True, stop=True)
                                j = kc - 4 * g
                                if j >= 0:
                                    nc.vector.tensor_tensor(
                                        st_ps[:], st_ps[:], mt[:, j, :],
                                        mybir.AluOpType.add)
                                e = asb.tile([P, TC], f32r, tag="E",
                                             name=f"e_{head}_{g}_{kc}")
                                nc.scalar.activation(
                                    e[:], st_ps[:],
                                    mybir.ActivationFunctionType.Exp)
                                # denominator partials on DVE (frees PE)
                                if kc == 1:
                                    nc.vector.tensor_tensor(
                                        es[:], e_prev[:], e[:],
                                        mybir.AluOpType.add)
                                elif kc > 1:
                                    nc.vector.tensor_tensor(
                                        es[:], es[:], e[:],
                                        mybir.AluOpType.add)
                                e_prev = e
                                nc.tensor.matmul(
                                    pv[:], vnat[:, kc, :], e[:],
                                    start=(kc == 0), stop=(kc == kmax - 1))
                            nc.tensor.matmul(d_ps[:], o_k[:], es[:],
                                             start=True, stop=True)
                            rd = asm_p.tile([1, TC], f32, tag="rd")
                            nc.vector.reciprocal(rd[:], d_ps[:])
                            rdr = asm_p.tile([1, TC], f32r, tag="rdr")
                            nc.scalar.copy(rdr[:], rd[:])
                            rb = rbp.tile([P, TC], f32, tag="rb")
                            nc.tensor.matmul(rb[:], o_m[:], rdr[:],
                                             start=True, stop=True)
                            rbs = asm_p.tile([P, TC], f32, tag="rbs")
                            nc.scalar.copy(rbs[:], rb[:])
                            nc.vector.tensor_tensor(
                                attnT[:, head, qsl], pv[:], rbs[:],
                                mybir.AluOpType.mult)

                # ---------------- phase 5: out = attn @ wo_shard -----------
                with tc.tile_pool(name="wo_sb", bufs=3) as wsb, \
                     tc.tile_pool(name="o_sb", bufs=4) as osb, \
                     tc.tile_pool(name="o_ps", bufs=4, space="PSUM") as ops:
                    for oc in range(NOC):
                        wot = wsb.tile([P, QH, TC], f32r, tag="wot")
                        nc.sync.dma_start(
                            wot[:],
                            wo_s[:, oc * TC:(oc + 1) * TC].rearrange(
                                "(hc p) n -> p hc n", p=P))
                        for tcn in range(NTC16):
                            o_ps = ops.tile([P, TC], f32, tag="o")
                            for hc in range(QH):
                                nc.tensor.matmul(
                                    o_ps[:],
                                    attnT[:, hc, tcn * P:(tcn + 1) * P],
                                    wot[:, hc, :],
                                    start=(hc == 0), stop=(hc == QH - 1))
                            ob = osb.tile([P, TC], f32, tag="ob")
                            nc.scalar.copy(ob[:], o_ps[:])
                            nc.gpsimd.dma_start(
                                part[tcn * P:(tcn + 1) * P,
                                     oc * TC:(oc + 1) * TC], ob[:])

    nc.compile()
    return nc


def _rope_tables(positions):
    pos = positions.astype(np.float64)
    inv_freq = 1.0 / (BASE ** (np.arange(HALF, dtype=np.float64) / HALF))
    freqs = pos[:, None] * inv_freq[None, :]          # [T, 64]
    cos = np.cos(freqs)
    sin = np.sin(freqs)
    cosT = np.concatenate([cos, cos], axis=1).T       # [128, T]
    sinT = np.concatenate([-sin, sin], axis=1).T      # sign folded
    return cosT.astype(np.float32), sinT.astype(np.float32)


def kernel(positions, hidden_states, wqkv, wo):
    global _COMPILED
    if _COMPILED is None:
        _COMPILED = _build()
    nc = _COMPILED

    scale = HD ** -0.5
    cosT, sinT = _rope_tables(positions)
    cosq = np.ascontiguousarray(cosT * scale)
    sinq = np.ascontiguousarray(sinT * scale)

    hidT = np.ascontiguousarray(hidden_states.T)

    # causal mask add-tiles for the diagonal blocks, ST layout [k, q]:
    # block j (k chunk 4g+j vs q group g): valid iff 128*j + kl <= ql
    kl = np.arange(P)[:, None]
    ql = np.arange(TC)[None, :]
    masks = np.stack(
        [np.where(P * j + kl <= ql, 0.0, NEG) for j in range(4)],
        axis=1).astype(np.float32)                    # [128, 4, 512]

    rperm = np.zeros((P, P), dtype=np.float32)
    for m in range(P):
        rperm[(m + HALF) % P, m] = 1.0                # out[m]=x[(m+64)%128]
    ident = np.eye(P, dtype=np.float32)
    ones_k = np.ones((P, 1), dtype=np.float32)
    ones_m = np.ones((1, P), dtype=np.float32)

    in_maps = []
    for r in range(NCORES):
        qc = slice(r * QCOLS, (r + 1) * QCOLS)
        kc = slice(NH * HD + r * HD, NH * HD + (r + 1) * HD)
        vc = slice((NH + NKV) * HD + r * HD, (NH + NKV) * HD + (r + 1) * HD)
        wqkv_s = np.ascontiguousarray(
            np.concatenate([wqkv[:, qc], wqkv[:, kc], wqkv[:, vc]], axis=1))
        wo_s = np.ascontiguousarray(wo[qc, :])
        in_maps.append({
            "hidT": hidT, "wqkv_s": wqkv_s, "wo_s": wo_s,
            "cosq": cosq, "sinq": sinq, "cosk": cosT, "sink": sinT,
            "masks": masks, "rperm": rperm, "ident": ident,
            "ones_k": ones_k, "ones_m": ones_m,
        })

    global _LAST_IN_MAPS
    _LAST_IN_MAPS = in_maps
    res = run_bass_kernel_spmd(nc, in_maps, list(range(NCORES)))
    out = res.results[0]["part"].astype(np.float64)
    for r in range(1, NCORES):
        out += res.results[r]["part"]
    return out.astype(np.float32)



# revision 7
# speedup vs baseline: 1.4193x; 1.4193x over previous
"""InternLM3 self-attention (prefill, GQA, RoPE) on 8 Trainium2 cores.

Tensor-parallel over heads: core r owns q heads 4r..4r+3 and kv head r
(wqkv column shards, wo row shards).  Each core computes its partial
output projection; the 8 partials are summed on the host.

v2 redesign vs the fp32r baseline:
  - all matmuls in bf16 (enables Fast Weight Load; fp32r disables it and
    costs ~125 ns/matmul of serialized LDWEIGHTS).
  - single software pipeline over the 4 token chunks: proj(t) -> rope(t)
    -> attention(g=t) -> out-proj(t), so PE never waits on a phase.
  - softmax denominator accumulated on PE (ones-vector matmul into PSUM)
    instead of a DVE chain; 1/d via reciprocal_approx_fast on the compact
    [1,512] tile, broadcast via gpsimd partition_broadcast (no more 4 us
    [1,512] DVE reciprocals / tiny 1x128 matmuls).
  - RoPE rotate-half via SBUF->SBUF DMA partition shift (no PE/PSUM).
  - q AND k both scaled by 128^-0.25-equivalent tables; exp() applies
    the compensating scale via its free affine input.
  - all inputs converted to bf16 on the host; partial output in bf16.
"""

import numpy as np
import ml_dtypes

import concourse.bass as bass
import concourse.bacc as bacc
import concourse.mybir as mybir
import concourse.tile as tile
from concourse.bass_utils import run_bass_kernel_spmd

T = 2048
H = 4096
NH = 32
NKV = 8
HD = 128
HALF = HD // 2
BASE = 1000000.0
NCORES = 8
QH = NH // NCORES            # 4 q heads per core
QCOLS = QH * HD              # 512
SH_COLS = QCOLS + 2 * HD     # 768 wqkv cols per core
NEG = -1e30

P = 128
TC = 512                     # token chunk
NT = T // TC                 # 4
NHC = H // P                 # 32 contraction chunks
NQC = SH_COLS // P           # 6 qkv col chunks (0-3 q heads, 4 k, 5 v)
NOC = H // TC                # 8 output col chunks

f32 = mybir.dt.float32
bf16 = mybir.dt.bfloat16

EXP_SCALE = float(np.sqrt(128.0))   # tables carry 128^-0.5 on q and k

_COMPILED = None


def _build():
    nc = bacc.Bacc("TRN2", target_bir_lowering=False, debug=False,
                   num_devices=NCORES)

    hidT = nc.dram_tensor("hidT", [H, T], bf16, kind="ExternalInput").ap()
    wqkv_s = nc.dram_tensor("wqkv_s", [H, SH_COLS], bf16,
                            kind="ExternalInput").ap()
    wo_s = nc.dram_tensor("wo_s", [QCOLS, H], bf16,
                          kind="ExternalInput").ap()
    cosq = nc.dram_tensor("cosq", [P, T], bf16, kind="ExternalInput").ap()
    sinq = nc.dram_tensor("sinq", [P, T], bf16, kind="ExternalInput").ap()
    masks = nc.dram_tensor("masks", [P, 4, TC], bf16,
                           kind="ExternalInput").ap()
    ident = nc.dram_tensor("ident", [P, P], bf16, kind="ExternalInput").ap()
    ones_k = nc.dram_tensor("ones_k", [P, 1], bf16,
                            kind="ExternalInput").ap()
    part = nc.dram_tensor("part", [T, H], bf16, kind="ExternalOutput").ap()

    with tile.TileContext(nc) as tc:
        with tc.tile_pool(name="keep", bufs=1) as keep, \
             tc.tile_pool(name="hid", bufs=5) as hidp, \
             tc.tile_pool(name="attn", bufs=2) as attp, \
             tc.tile_pool(name="rot", bufs=2) as rotp, \
             tc.tile_pool(name="e", bufs=4) as ep, \
             tc.tile_pool(name="rd", bufs=1) as rdp, \
             tc.tile_pool(name="rbs", bufs=2) as rbsp, \
             tc.tile_pool(name="ob", bufs=3) as obp, \
             tc.tile_pool(name="qps", bufs=2, space="PSUM") as qpsp, \
             tc.tile_pool(name="acc", bufs=1, space="PSUM") as accp, \
             tc.tile_pool(name="st", bufs=2, space="PSUM") as stp, \
             tc.tile_pool(name="ops", bufs=2, space="PSUM") as opsp:

            # ---- long-lived SBUF ----
            wq = keep.tile([P, NHC, SH_COLS], bf16)        # 48 KB/part
            wo_r = keep.tile([P, QH, H], bf16)             # 32 KB/part
            qkvT = keep.tile([P, NQC, T], bf16)            # 24 KB/part
            ct = keep.tile([P, T], bf16, tag="cosq_t")     # 4 KB
            st_t = keep.tile([P, T], bf16, tag="sinq_t")   # 4 KB
            mt = keep.tile([P, 4, TC], bf16, tag="masks_t")  # 4 KB
            vnat = keep.tile([P, T // P, P], bf16, tag="vnat")  # 4 KB
            idt = keep.tile([P, P], bf16, tag="ident_t")
            o_k = keep.tile([P, 1], bf16, tag="ones_k_t")

            # constants / weights prefetch
            nc.sync.dma_start(ct[:], cosq[:])
            nc.sync.dma_start(st_t[:], sinq[:])
            nc.sync.dma_start(mt[:], masks[:])
            nc.sync.dma_start(idt[:], ident[:])
            nc.sync.dma_start(o_k[:], ones_k[:])
            for h in range(NHC):
                nc.sync.dma_start(wq[:, h, :], wqkv_s[h * P:(h + 1) * P, :])
            for hc in range(QH):
                for oc in range(4):
                    nc.gpsimd.dma_start(
                        wo_r[:, hc, oc * 1024:(oc + 1) * 1024],
                        wo_s[hc * P:(hc + 1) * P, oc * 1024:(oc + 1) * 1024])

            for t in range(NT):
                tsl = slice(t * TC, (t + 1) * TC)

                # ---- hidT chunk streamed in quarters, multi-buffered
                QTR = NHC // 4
                hid_q = []
                for qi in range(4):
                    hq = hidp.tile([P, QTR, TC], bf16, tag="hid",
                                   name=f"hid_{t}_{qi}")
                    for h in range(QTR):
                        hh = qi * QTR + h
                        nc.sync.dma_start(
                            hq[:, h, :], hidT[hh * P:(hh + 1) * P, tsl])
                    hid_q.append(hq)

                # ---- phase 1: qkv^T chunk = wqkv^T @ hidden^T ----
                for c in range(NQC):
                    qps = qpsp.tile([P, TC], f32, tag="qps",
                                    name=f"qps_{t}_{c}")
                    for h in range(NHC):
                        nc.tensor.matmul(
                            qps[:], wq[:, h, c * P:(c + 1) * P],
                            hid_q[h // QTR][:, h % QTR, :],
                            start=(h == 0), stop=(h == NHC - 1))
                    nc.scalar.copy(qkvT[:, c, tsl], qps[:])

                # ---- v transpose for this chunk (PE, shares st slots) ----
                for j in range(TC // P):
                    kc = t * (TC // P) + j
                    tp = stp.tile([P, TC], bf16, tag="st", name=f"vt_{kc}")
                    nc.tensor.transpose(
                        tp[:, 0:P], qkvT[:, 5, kc * P:(kc + 1) * P], idt[:])
                    nc.scalar.copy(vnat[:, kc, :], tp[:, 0:P])

                # ---- phase 2: RoPE on q heads and k head (in place) ----
                for idx in range(QH + 1):
                    x = qkvT[:, idx, tsl]
                    rot = rotp.tile([P, TC], bf16, tag="rot",
                                    name=f"rot_{t}_{idx}")
                    nc.gpsimd.dma_start(rot[0:HALF, :],
                                        qkvT[HALF:P, idx, tsl])
                    nc.gpsimd.dma_start(rot[HALF:P, :],
                                        qkvT[0:HALF, idx, tsl])
                    nc.vector.tensor_tensor(
                        rot[:], rot[:], st_t[:, tsl], mybir.AluOpType.mult)
                    nc.vector.tensor_tensor(
                        x, x, ct[:, tsl], mybir.AluOpType.mult)
                    nc.vector.tensor_tensor(
                        x, x, rot[:], mybir.AluOpType.add)

                # ---- phase 4: causal attention, q group g == t ----
                attnT = attp.tile([P, QH, TC], bf16, tag="attnT",
                                  name=f"attnT_{t}")
                kmax = 4 * (t + 1)
                for head in range(QH):
                    d_ps = accp.tile([1, TC], f32, tag="d",
                                     name=f"d_{t}_{head}")
                    pv = accp.tile([P, TC], f32, tag="pv",
                                   name=f"pv_{t}_{head}")
                    e_prev = None
                    for kc in range(kmax):
                        st_ps = stp.tile([P, TC], f32, tag="st",
                                         name=f"st_{t}_{head}_{kc}")
                        nc.tensor.matmul(
                            st_ps[:],
                            qkvT[:, QH, kc * P:(kc + 1) * P],
                            qkvT[:, head, tsl],
                            start=True, stop=True)
                        # previous chunk's dsum+pv land between scores
                        if e_prev is not None:
                            pkc, pe = e_prev
                            nc.tensor.matmul(d_ps[:], o_k[:], pe[:],
                                             start=(pkc == 0), stop=False)
                            nc.tensor.matmul(pv[:], vnat[:, pkc, :], pe[:],
                                             start=(pkc == 0), stop=False)
                        j = kc - 4 * t
                        if j >= 0:
                            nc.vector.tensor_tensor(
                                st_ps[:], st_ps[:], mt[:, j, :],
                                mybir.AluOpType.add)
                        e = ep.tile([P, TC], bf16, tag="E",
                                    name=f"e_{t}_{head}_{kc}")
                        nc.scalar.activation(
                            e[:], st_ps[:],
                            mybir.ActivationFunctionType.Exp,
                            scale=EXP_SCALE)
                        e_prev = (kc, e)
                    pkc, pe = e_prev
                    nc.tensor.matmul(d_ps[:], o_k[:], pe[:],
                                     start=(pkc == 0), stop=True)
                    nc.tensor.matmul(pv[:], vnat[:, pkc, :], pe[:],
                                     start=(pkc == 0), stop=True)
                    rd = rdp.tile([1, TC], f32, tag="rd",
                                  name=f"rd_{t}_{head}")
                    nc.vector.reciprocal_approx_fast(rd[:], d_ps[:])
                    rbs = rbsp.tile([P, TC], f32, tag="rbs",
                                    name=f"rbs_{t}_{head}")
                    nc.gpsimd.partition_broadcast(rbs[:], rd[:])
                    nc.vector.tensor_tensor(
                        attnT[:, head, :], pv[:], rbs[:],
                        mybir.AluOpType.mult)

                # ---- phase 5: out chunk = attn(t) @ wo_shard ----
                for oc in range(NOC):
                    for tcn in range(TC // P):
                        o_ps = opsp.tile([P, TC], f32, tag="o",
                                         name=f"o_{t}_{oc}_{tcn}")
                        for hc in range(QH):
                            nc.tensor.matmul(
                                o_ps[:],
                                attnT[:, hc, tcn * P:(tcn + 1) * P],
                                wo_r[:, hc, oc * TC:(oc + 1) * TC],
                                start=(hc == 0), stop=(hc == QH - 1))
                        ob = obp.tile([P, TC], bf16, tag="ob",
                                      name=f"ob_{t}_{oc}_{tcn}")
                        if (oc + tcn) % 2 == 0:
                            nc.scalar.copy(ob[:], o_ps[:])
                        else:
                            nc.vector.tensor_copy(ob[:], o_ps[:])
                        nc.gpsimd.dma_start(
                            part[t * TC + tcn * P:t * TC + (tcn + 1) * P,
                                 oc * TC:(oc + 1) * TC], ob[:])

    nc.compile()
    return nc


def _rope_tables(positions):
    pos = positions.astype(np.float64)
    inv_freq = 1.0 / (BASE ** (np.arange(HALF, dtype=np.float64) / HALF))
    freqs = pos[:, None] * inv_freq[None, :]          # [T, 64]
    cos = np.cos(freqs)
    sin = np.sin(freqs)
    cosT = np.concatenate([cos, cos], axis=1).T       # [128, T]
    sinT = np.concatenate([-sin, sin], axis=1).T      # sign folded
    return cosT, sinT


def kernel(positions, hidden_states, wqkv, wo):
    global _COMPILED
    if _COMPILED is None:
        _COMPILED = _build()
    nc = _COMPILED

    s = 128.0 ** -0.5                                 # per-side score scale
    cosT, sinT = _rope_tables(positions)
    cosq = np.ascontiguousarray(cosT * s).astype(ml_dtypes.bfloat16)
    sinq = np.ascontiguousarray(sinT * s).astype(ml_dtypes.bfloat16)

    hidT = np.ascontiguousarray(hidden_states.T).astype(ml_dtypes.bfloat16)

    # causal mask add-tiles, ST layout [k, q], diagonal blocks only
    kl = np.arange(P)[:, None]
    ql = np.arange(TC)[None, :]
    masks = np.stack(
        [np.where(P * j + kl <= ql, 0.0, NEG) for j in range(4)],
        axis=1).astype(ml_dtypes.bfloat16)            # [128, 4, 512]

    ident = np.eye(P, dtype=np.float32).astype(ml_dtypes.bfloat16)
    ones_k = np.ones((P, 1), dtype=np.float32).astype(ml_dtypes.bfloat16)

    in_maps = []
    for r in range(NCORES):
        qc = slice(r * QCOLS, (r + 1) * QCOLS)
        kc = slice(NH * HD + r * HD, NH * HD + (r + 1) * HD)
        vc = slice((NH + NKV) * HD + r * HD, (NH + NKV) * HD + (r + 1) * HD)
        wqkv_s = np.ascontiguousarray(
            np.concatenate([wqkv[:, qc], wqkv[:, kc], wqkv[:, vc]],
                           axis=1)).astype(ml_dtypes.bfloat16)
        wo_s = np.ascontiguousarray(wo[qc, :]).astype(ml_dtypes.bfloat16)
        in_maps.append({
            "hidT": hidT, "wqkv_s": wqkv_s, "wo_s": wo_s,
            "cosq": cosq, "sinq": sinq, "masks": masks,
            "ident": ident, "ones_k": ones_k,
        })

    global _LAST_IN_MAPS
    _LAST_IN_MAPS = in_maps
    res = run_bass_kernel_spmd(nc, in_maps, list(range(NCORES)))
    out = res.results[0]["part"].astype(np.float64)
    for r in range(1, NCORES):
        out += res.results[r]["part"].astype(np.float64)
    return out.astype(np.float32)


# revision 8
# speedup vs baseline: 1.5785x; 1.1122x over previous
"""InternLM3 self-attention (prefill, GQA, RoPE) on 8 Trainium2 cores.

Tensor-parallel over heads: core r owns q heads 4r..4r+3 and kv head r
(wqkv column shards, wo row shards).  Each core computes its partial
output projection; the 8 partials are summed on the host.

v3 design:
  - all matmuls bf16 (Fast Weight Load on; fp32r would serialize
    LDWEIGHTS at ~125 ns/matmul).
  - one software pipeline over the 4 token chunks: proj(t) -> rope(t)
    -> attention(g=t) -> out-proj(t); per-tile semaphores overlap the
    phases across engines.
  - softmax denominator accumulated on PE with an all-ones [128,128]
    stationary (result replicated across partitions), so 1/d is a single
    reciprocal_approx_fast on [128,512] and the normalizing multiply
    needs no broadcast at all.
  - scores/pv/denominator matmuls run 2 chunks behind the scores stream
    so PSUM bank recycling at head boundaries never stalls the PE.
  - RoPE rotate-half via SBUF->SBUF DMA partition shift; k head is
    roped first so attention never waits on it.
  - q and k both carry 128^-0.5 in the rope tables; exp() applies the
    compensating sqrt(128) via its free affine scale.
  - batched weight DMAs (descriptors fan out across all 16 queues);
    wo load is emitted late so it never delays the first projection.
"""

import numpy as np
import ml_dtypes

import concourse.bass as bass
import concourse.bacc as bacc
import concourse.mybir as mybir
import concourse.tile as tile
from concourse.bass_utils import run_bass_kernel_spmd

T = 2048
H = 4096
NH = 32
NKV = 8
HD = 128
HALF = HD // 2
BASE = 1000000.0
NCORES = 8
QH = NH // NCORES            # 4 q heads per core
QCOLS = QH * HD              # 512
SH_COLS = QCOLS + 2 * HD     # 768 wqkv cols per core
NEG = -1e30

P = 128
TC = 512                     # token chunk
NT = T // TC                 # 4
NHC = H // P                 # 32 contraction chunks
NQC = SH_COLS // P           # 6 qkv col chunks (0-3 q heads, 4 k, 5 v)
NOC = H // TC                # 8 output col chunks
LAG = 2                      # chunks the pv/denominator stream trails by

f32 = mybir.dt.float32
bf16 = mybir.dt.bfloat16

EXP_SCALE = float(np.sqrt(128.0))   # tables carry 128^-0.5 on q and k

_COMPILED = None


def _build():
    nc = bacc.Bacc("TRN2", target_bir_lowering=False, debug=False,
                   num_devices=NCORES)

    hidT = nc.dram_tensor("hidT", [H, T], bf16, kind="ExternalInput").ap()
    wqkv_s = nc.dram_tensor("wqkv_s", [H, SH_COLS], bf16,
                            kind="ExternalInput").ap()
    wo_s = nc.dram_tensor("wo_s", [QCOLS, H], bf16,
                          kind="ExternalInput").ap()
    cosq = nc.dram_tensor("cosq", [P, T], bf16, kind="ExternalInput").ap()
    sinq = nc.dram_tensor("sinq", [P, T], bf16, kind="ExternalInput").ap()
    masks = nc.dram_tensor("masks", [P, 4, TC], bf16,
                           kind="ExternalInput").ap()
    ident = nc.dram_tensor("ident", [P, P], bf16, kind="ExternalInput").ap()
    onesm = nc.dram_tensor("onesm", [P, P], bf16, kind="ExternalInput").ap()
    part = nc.dram_tensor("part", [T, H], bf16, kind="ExternalOutput").ap()

    with tile.TileContext(nc) as tc:
        with tc.tile_pool(name="keep", bufs=1) as keep, \
             tc.tile_pool(name="hid", bufs=5) as hidp, \
             tc.tile_pool(name="attn", bufs=2) as attp, \
             tc.tile_pool(name="rot", bufs=2) as rotp, \
             tc.tile_pool(name="e", bufs=6) as ep, \
             tc.tile_pool(name="rbs", bufs=2) as rbsp, \
             tc.tile_pool(name="ob", bufs=4) as obp, \
             tc.tile_pool(name="qps", bufs=2, space="PSUM") as qpsp, \
             tc.tile_pool(name="acc", bufs=1, space="PSUM") as accp, \
             tc.tile_pool(name="st", bufs=2, space="PSUM") as stp, \
             tc.tile_pool(name="ops", bufs=2, space="PSUM") as opsp:

            # ---- long-lived SBUF ----
            wq = keep.tile([P, NHC, SH_COLS], bf16)        # 48 KB/part
            wo_r = keep.tile([P, QH, H], bf16)             # 32 KB/part
            qkvT = keep.tile([P, NQC, T], bf16)            # 24 KB/part
            ct = keep.tile([P, T], bf16, tag="cosq_t")     # 4 KB
            st_t = keep.tile([P, T], bf16, tag="sinq_t")   # 4 KB
            mt = keep.tile([P, 4, TC], bf16, tag="masks_t")  # 4 KB
            vnat = keep.tile([P, T // P, P], bf16, tag="vnat")  # 4 KB
            idt = keep.tile([P, P], bf16, tag="ident_t")
            o_m = keep.tile([P, P], bf16, tag="onesm_t")

            # hid(0) + wq first: these gate the very first matmuls.
            QTR = NHC // 4
            hid_t0 = []
            for qi in range(4):
                hq = hidp.tile([P, QTR, TC], bf16, tag="hid",
                               name=f"hid_0_{qi}")
                nc.sync.dma_start(
                    hq[:],
                    hidT[qi * QTR * P:(qi + 1) * QTR * P, 0:TC].rearrange(
                        "(h p) n -> p h n", p=P))
                hid_t0.append(hq)
            nc.sync.dma_start(
                wq[:], wqkv_s[:].rearrange("(h p) c -> p h c", p=P))
            nc.scalar.dma_start(ct[:], cosq[:])
            nc.scalar.dma_start(st_t[:], sinq[:])
            nc.scalar.dma_start(mt[:], masks[:])
            nc.scalar.dma_start(idt[:], ident[:])
            nc.scalar.dma_start(o_m[:], onesm[:])

            for t in range(NT):
                tsl = slice(t * TC, (t + 1) * TC)

                if t == 0:
                    hid_q = hid_t0
                else:
                    hid_q = []
                    for qi in range(4):
                        hq = hidp.tile([P, QTR, TC], bf16, tag="hid",
                                       name=f"hid_{t}_{qi}")
                        nc.sync.dma_start(
                            hq[:],
                            hidT[qi * QTR * P:(qi + 1) * QTR * P,
                                 tsl].rearrange("(h p) n -> p h n", p=P))
                        hid_q.append(hq)

                # ---- phase 1: qkv^T chunk = wqkv^T @ hidden^T ----
                # k and v first so rope(k) / v-transpose overlap the q cols.
                def proj_col(c):
                    qps = qpsp.tile([P, TC], f32, tag="qps",
                                    name=f"qps_{t}_{c}")
                    for h in range(NHC):
                        nc.tensor.matmul(
                            qps[:], wq[:, h, c * P:(c + 1) * P],
                            hid_q[h // QTR][:, h % QTR, :],
                            start=(h == 0), stop=(h == NHC - 1))
                    nc.vector.tensor_copy(qkvT[:, c, tsl], qps[:])

                def rope_col(idx):
                    x = qkvT[:, idx, tsl]
                    rot = rotp.tile([P, TC], bf16, tag="rot",
                                    name=f"rot_{t}_{idx}")
                    nc.gpsimd.dma_start(rot[0:HALF, :],
                                        qkvT[HALF:P, idx, tsl])
                    nc.gpsimd.dma_start(rot[HALF:P, :],
                                        qkvT[0:HALF, idx, tsl])
                    nc.vector.tensor_tensor(
                        rot[:], rot[:], st_t[:, tsl], mybir.AluOpType.mult)
                    nc.vector.tensor_tensor(
                        x, x, ct[:, tsl], mybir.AluOpType.mult)
                    nc.vector.tensor_tensor(
                        x, x, rot[:], mybir.AluOpType.add)

                proj_col(4)                      # k
                proj_col(5)                      # v
                rope_col(QH)                     # rope k immediately
                for j in range(TC // P):         # v transpose
                    kc = t * (TC // P) + j
                    tp = stp.tile([P, TC], bf16, tag="st", name=f"vt_{kc}")
                    nc.tensor.transpose(
                        tp[:, 0:P], qkvT[:, 5, kc * P:(kc + 1) * P], idt[:])
                    nc.scalar.copy(vnat[:, kc, :], tp[:, 0:P])
                for c in range(QH):              # q heads + their rope
                    proj_col(c)
                    rope_col(c)

                if t == 0:
                    # wo load, deferred so it never races the hot path
                    for hc in range(QH):
                        nc.sync.dma_start(wo_r[:, hc, :],
                                          wo_s[hc * P:(hc + 1) * P, :])

                # ---- phase 4: causal attention, q group g == t ----
                attnT = attp.tile([P, QH, TC], bf16, tag="attnT",
                                  name=f"attnT_{t}")
                kmax = 4 * (t + 1)
                for head in range(QH):
                    d_rep = accp.tile([P, TC], f32, tag="d",
                                      name=f"d_{t}_{head}")
                    pv = accp.tile([P, TC], f32, tag="pv",
                                   name=f"pv_{t}_{head}")
                    es = []

                    def drain_one():
                        pkc, pe = es.pop(0)
                        nc.tensor.matmul(d_rep[:], o_m[:], pe[:],
                                         start=(pkc == 0),
                                         stop=(pkc == kmax - 1))
                        nc.tensor.matmul(pv[:], vnat[:, pkc, :], pe[:],
                                         start=(pkc == 0),
                                         stop=(pkc == kmax - 1))

                    for kc in range(kmax):
                        st_ps = stp.tile([P, TC], f32, tag="st",
                                         name=f"st_{t}_{head}_{kc}")
                        nc.tensor.matmul(
                            st_ps[:],
                            qkvT[:, QH, kc * P:(kc + 1) * P],
                            qkvT[:, head, tsl],
                            start=True, stop=True)
                        if len(es) >= LAG:
                            drain_one()
                        j = kc - 4 * t
                        if j >= 0:
                            nc.vector.tensor_tensor(
                                st_ps[:], st_ps[:], mt[:, j, :],
                                mybir.AluOpType.add)
                        e = ep.tile([P, TC], bf16, tag="E",
                                    name=f"e_{t}_{head}_{kc}")
                        nc.scalar.activation(
                            e[:], st_ps[:],
                            mybir.ActivationFunctionType.Exp,
                            scale=EXP_SCALE)
                        es.append((kc, e))
                    while es:
                        drain_one()
                    rbs = rbsp.tile([P, TC], f32, tag="rbs",
                                    name=f"rbs_{t}_{head}")
                    nc.vector.reciprocal_approx_fast(rbs[:], d_rep[:])
                    nc.vector.tensor_tensor(
                        attnT[:, head, :], pv[:], rbs[:],
                        mybir.AluOpType.mult)

                # ---- phase 5: out chunk = attn(t) @ wo_shard ----
                for oc in range(NOC):
                    for tcn in range(TC // P):
                        o_ps = opsp.tile([P, TC], f32, tag="o",
                                         name=f"o_{t}_{oc}_{tcn}")
                        for hc in range(QH):
                            nc.tensor.matmul(
                                o_ps[:],
                                attnT[:, hc, tcn * P:(tcn + 1) * P],
                                wo_r[:, hc, oc * TC:(oc + 1) * TC],
                                start=(hc == 0), stop=(hc == QH - 1))
                        ob = obp.tile([P, TC], bf16, tag="ob",
                                      name=f"ob_{t}_{oc}_{tcn}")
                        if (oc + tcn) % 2 == 0:
                            nc.scalar.copy(ob[:], o_ps[:])
                        else:
                            nc.vector.tensor_copy(ob[:], o_ps[:])
                        nc.gpsimd.dma_start(
                            part[t * TC + tcn * P:t * TC + (tcn + 1) * P,
                                 oc * TC:(oc + 1) * TC], ob[:])

    nc.compile()
    return nc


def _rope_tables(positions):
    pos = positions.astype(np.float64)
    inv_freq = 1.0 / (BASE ** (np.arange(HALF, dtype=np.float64) / HALF))
    freqs = pos[:, None] * inv_freq[None, :]          # [T, 64]
    cos = np.cos(freqs)
    sin = np.sin(freqs)
    cosT = np.concatenate([cos, cos], axis=1).T       # [128, T]
    sinT = np.concatenate([-sin, sin], axis=1).T      # sign folded
    return cosT, sinT


def kernel(positions, hidden_states, wqkv, wo):
    global _COMPILED
    if _COMPILED is None:
        _COMPILED = _build()
    nc = _COMPILED

    s = 128.0 ** -0.5                                 # per-side score scale
    cosT, sinT = _rope_tables(positions)
    cosq = np.ascontiguousarray(cosT * s).astype(ml_dtypes.bfloat16)
    sinq = np.ascontiguousarray(sinT * s).astype(ml_dtypes.bfloat16)

    hidT = np.ascontiguousarray(hidden_states.T).astype(ml_dtypes.bfloat16)

    # causal mask add-tiles, ST layout [k, q], diagonal blocks only
    kl = np.arange(P)[:, None]
    ql = np.arange(TC)[None, :]
    masks = np.stack(
        [np.where(P * j + kl <= ql, 0.0, NEG) for j in range(4)],
        axis=1).astype(ml_dtypes.bfloat16)            # [128, 4, 512]

    ident = np.eye(P, dtype=np.float32).astype(ml_dtypes.bfloat16)
    onesm = np.ones((P, P), dtype=np.float32).astype(ml_dtypes.bfloat16)

    in_maps = []
    for r in range(NCORES):
        qc = slice(r * QCOLS, (r + 1) * QCOLS)
        kc = slice(NH * HD + r * HD, NH * HD + (r + 1) * HD)
        vc = slice((NH + NKV) * HD + r * HD, (NH + NKV) * HD + (r + 1) * HD)
        wqkv_s = np.ascontiguousarray(
            np.concatenate([wqkv[:, qc], wqkv[:, kc], wqkv[:, vc]],
                           axis=1)).astype(ml_dtypes.bfloat16)
        wo_s = np.ascontiguousarray(wo[qc, :]).astype(ml_dtypes.bfloat16)
        in_maps.append({
            "hidT": hidT, "wqkv_s": wqkv_s, "wo_s": wo_s,
            "cosq": cosq, "sinq": sinq, "masks": masks,
            "ident": ident, "onesm": onesm,
        })

    global _LAST_IN_MAPS
    _LAST_IN_MAPS = in_maps
    res = run_bass_kernel_spmd(nc, in_maps, list(range(NCORES)))
    out = res.results[0]["part"].astype(np.float64)
    for r in range(1, NCORES):
        out += res.results[r]["part"].astype(np.float64)
    return out.astype(np.float32)


# revision 16
# speedup vs baseline: 1.6060x; 1.0174x over previous
"""InternLM3 self-attention (prefill, GQA, RoPE) on 8 Trainium2 cores.

Tensor-parallel over heads: core r owns q heads 4r..4r+3 and kv head r
(wqkv column shards, wo row shards).  Each core computes its partial
output projection; the 8 partials are summed on the host.

v3 design:
  - all matmuls bf16 (Fast Weight Load on; fp32r would serialize
    LDWEIGHTS at ~125 ns/matmul).
  - one software pipeline over the 4 token chunks: proj(t) -> rope(t)
    -> attention(g=t) -> out-proj(t); per-tile semaphores overlap the
    phases across engines.
  - softmax denominator accumulated on PE with an all-ones [128,128]
    stationary (result replicated across partitions), so 1/d is a single
    reciprocal_approx_fast on [128,512] and the normalizing multiply
    needs no broadcast at all.
  - scores/pv/denominator matmuls run 2 chunks behind the scores stream
    so PSUM bank recycling at head boundaries never stalls the PE.
  - RoPE rotate-half via SBUF->SBUF DMA partition shift; k head is
    roped first so attention never waits on it.
  - q and k both carry 128^-0.5 in the rope tables; exp() applies the
    compensating sqrt(128) via its free affine scale.
  - batched weight DMAs (descriptors fan out across all 16 queues);
    wo load is emitted late so it never delays the first projection.
"""

import numpy as np
import ml_dtypes

import concourse.bass as bass
import concourse.bacc as bacc
import concourse.mybir as mybir
import concourse.tile as tile
from concourse.bass_utils import run_bass_kernel_spmd

T = 2048
H = 4096
NH = 32
NKV = 8
HD = 128
HALF = HD // 2
BASE = 1000000.0
NCORES = 8
QH = NH // NCORES            # 4 q heads per core
QCOLS = QH * HD              # 512
SH_COLS = QCOLS + 2 * HD     # 768 wqkv cols per core
NEG = -1e30

P = 128
TC = 512                     # token chunk
NT = T // TC                 # 4
NHC = H // P                 # 32 contraction chunks
NQC = SH_COLS // P           # 6 qkv col chunks (0-3 q heads, 4 k, 5 v)
NOC = H // TC                # 8 output col chunks
LAG = 2                      # chunks the pv/denominator stream trails by

f32 = mybir.dt.float32
bf16 = mybir.dt.bfloat16

EXP_SCALE = float(np.sqrt(128.0))   # tables carry 128^-0.5 on q and k

_COMPILED = None


def _build():
    nc = bacc.Bacc("TRN2", target_bir_lowering=False, debug=False,
                   num_devices=NCORES)

    hidT = nc.dram_tensor("hidT", [H, T], bf16, kind="ExternalInput").ap()
    wqkv_s = nc.dram_tensor("wqkv_s", [H, SH_COLS], bf16,
                            kind="ExternalInput").ap()
    wo_s = nc.dram_tensor("wo_s", [QCOLS, H], bf16,
                          kind="ExternalInput").ap()
    cosq = nc.dram_tensor("cosq", [P, T], bf16, kind="ExternalInput").ap()
    sinq = nc.dram_tensor("sinq", [P, T], bf16, kind="ExternalInput").ap()
    masks = nc.dram_tensor("masks", [P, P], bf16,
                           kind="ExternalInput").ap()
    ident = nc.dram_tensor("ident", [P, P], bf16, kind="ExternalInput").ap()
    onesm = nc.dram_tensor("onesm", [P, P], bf16, kind="ExternalInput").ap()
    part = nc.dram_tensor("part", [T, H], bf16, kind="ExternalOutput").ap()

    with tile.TileContext(nc) as tc:
        with tc.tile_pool(name="keep", bufs=1) as keep, \
             tc.tile_pool(name="hid", bufs=5) as hidp, \
             tc.tile_pool(name="attn", bufs=2) as attp, \
             tc.tile_pool(name="rot", bufs=2) as rotp, \
             tc.tile_pool(name="e", bufs=6) as ep, \
             tc.tile_pool(name="rbs", bufs=2) as rbsp, \
             tc.tile_pool(name="ob", bufs=4) as obp, \
             tc.tile_pool(name="ps", bufs=4, space="PSUM") as psp, \
             tc.tile_pool(name="acc", bufs=1, space="PSUM") as accp, \
             tc.tile_pool(name="ops", bufs=2, space="PSUM") as opsp:

            # ---- long-lived SBUF ----
            wq = keep.tile([P, NHC, SH_COLS], bf16)        # 48 KB/part
            wo_r = keep.tile([P, QH, H], bf16)             # 32 KB/part
            qkvT = keep.tile([P, NQC, T], bf16)            # 24 KB/part
            ct = keep.tile([P, T], bf16, tag="cosq_t")     # 4 KB
            st_t = keep.tile([P, T], bf16, tag="sinq_t")   # 4 KB
            mt = keep.tile([P, P], bf16, tag="masks_t")    # staircase
            vnat = keep.tile([P, T // P, P], bf16, tag="vnat")  # 4 KB
            idt = keep.tile([P, P], bf16, tag="ident_t")
            o_m = keep.tile([P, P], bf16, tag="onesm_t")

            # hid(0) + wq first: these gate the very first matmuls.
            QTR = NHC // 4
            hid_t0 = []
            for qi in range(4):
                hq = hidp.tile([P, QTR, TC], bf16, tag="hid",
                               name=f"hid_0_{qi}")
                nc.sync.dma_start(
                    hq[:],
                    hidT[qi * QTR * P:(qi + 1) * QTR * P, 0:TC].rearrange(
                        "(h p) n -> p h n", p=P))
                hid_t0.append(hq)
            for c in (4, 5, 0, 1, 2, 3):      # proj column order
                nc.sync.dma_start(
                    wq[:, :, c * P:(c + 1) * P],
                    wqkv_s[:, c * P:(c + 1) * P].rearrange(
                        "(h p) c -> p h c", p=P))
            nc.scalar.dma_start(ct[:], cosq[:])
            nc.scalar.dma_start(st_t[:], sinq[:])
            nc.scalar.dma_start(mt[:], masks[:])
            nc.scalar.dma_start(idt[:], ident[:])
            nc.scalar.dma_start(o_m[:], onesm[:])

            for t in range(NT):
                tsl = slice(t * TC, (t + 1) * TC)

                if t == 0:
                    hid_q = hid_t0
                else:
                    hid_q = []
                    for qi in range(4):
                        hq = hidp.tile([P, QTR, TC], bf16, tag="hid",
                                       name=f"hid_{t}_{qi}")
                        nc.sync.dma_start(
                            hq[:],
                            hidT[qi * QTR * P:(qi + 1) * QTR * P,
                                 tsl].rearrange("(h p) n -> p h n", p=P))
                        hid_q.append(hq)

                # ---- phase 1: qkv^T chunk = wqkv^T @ hidden^T ----
                # k and v first so rope(k) / v-transpose overlap the q cols.
                def proj_col(c):
                    qps = psp.tile([P, TC], f32, tag="ps",
                                   name=f"qps_{t}_{c}")
                    for h in range(NHC):
                        nc.tensor.matmul(
                            qps[:], wq[:, h, c * P:(c + 1) * P],
                            hid_q[h // QTR][:, h % QTR, :],
                            start=(h == 0), stop=(h == NHC - 1))
                    nc.vector.tensor_copy(qkvT[:, c, tsl], qps[:])

                def rope_col(idx):
                    x = qkvT[:, idx, tsl]
                    rot = rotp.tile([P, TC], bf16, tag="rot",
                                    name=f"rot_{t}_{idx}")
                    nc.gpsimd.dma_start(rot[0:HALF, :],
                                        qkvT[HALF:P, idx, tsl])
                    nc.gpsimd.dma_start(rot[HALF:P, :],
                                        qkvT[0:HALF, idx, tsl])
                    nc.vector.tensor_tensor(
                        rot[:], rot[:], st_t[:, tsl], mybir.AluOpType.mult)
                    nc.vector.tensor_tensor(
                        x, x, ct[:, tsl], mybir.AluOpType.mult)
                    nc.vector.tensor_tensor(
                        x, x, rot[:], mybir.AluOpType.add)

                proj_col(4)                      # k
                proj_col(5)                      # v
                rope_col(QH)                     # rope k immediately
                for j in range(TC // P):         # v transpose
                    kc = t * (TC // P) + j
                    tp = psp.tile([P, TC], bf16, tag="ps", name=f"vt_{kc}")
                    nc.tensor.transpose(
                        tp[:, 0:P], qkvT[:, 5, kc * P:(kc + 1) * P], idt[:])
                    nc.scalar.copy(vnat[:, kc, :], tp[:, 0:P])
                for c in range(QH):              # q heads + their rope
                    proj_col(c)
                    rope_col(c)

                if t == 0:
                    # wo load, deferred so it never races the hot path
                    for hc in range(QH):
                        nc.sync.dma_start(wo_r[:, hc, :],
                                          wo_s[hc * P:(hc + 1) * P, :])

                # ---- phase 4: causal attention, q group g == t ----
                attnT = attp.tile([P, QH, TC], bf16, tag="attnT",
                                  name=f"attnT_{t}")
                kmax = 4 * (t + 1)
                for head in range(QH):
                    d_rep = accp.tile([P, TC], f32, tag="d",
                                      name=f"d_{t}_{head}")
                    pv = accp.tile([P, TC], f32, tag="pv",
                                   name=f"pv_{t}_{head}")
                    es = []

                    def drain_one():
                        pkc, pe = es.pop(0)
                        nc.tensor.matmul(d_rep[:], o_m[:], pe[:],
                                         start=(pkc == 0),
                                         stop=(pkc == kmax - 1))
                        nc.tensor.matmul(pv[:], vnat[:, pkc, :], pe[:],
                                         start=(pkc == 0),
                                         stop=(pkc == kmax - 1))

                    for kc in range(kmax):
                        st_ps = psp.tile([P, TC], f32, tag="ps",
                                         name=f"st_{t}_{head}_{kc}")
                        nc.tensor.matmul(
                            st_ps[:],
                            qkvT[:, QH, kc * P:(kc + 1) * P],
                            qkvT[:, head, tsl],
                            start=True, stop=True)
                        if len(es) >= LAG:
                            drain_one()
                        j = kc - 4 * t
                        e = ep.tile([P, TC], bf16, tag="E",
                                    name=f"e_{t}_{head}_{kc}")
                        if j >= 0:
                            # diagonal block: cols < 128j fully masked,
                            # staircase only in cols [128j, 128j+128)
                            nc.vector.tensor_tensor(
                                st_ps[:, j * P:(j + 1) * P],
                                st_ps[:, j * P:(j + 1) * P], mt[:],
                                mybir.AluOpType.add)
                            if j > 0:
                                nc.gpsimd.memset(e[:, 0:j * P], 0)
                            nc.scalar.activation(
                                e[:, j * P:], st_ps[:, j * P:],
                                mybir.ActivationFunctionType.Exp,
                                scale=EXP_SCALE)
                        else:
                            nc.scalar.activation(
                                e[:], st_ps[:],
                                mybir.ActivationFunctionType.Exp,
                                scale=EXP_SCALE)
                        es.append((kc, e))
                    while es:
                        drain_one()
                    rbs = rbsp.tile([P, TC], f32, tag="rbs",
                                    name=f"rbs_{t}_{head}")
                    nc.vector.reciprocal_approx_fast(rbs[:], d_rep[:])
                    nc.vector.tensor_tensor(
                        attnT[:, head, :], pv[:], rbs[:],
                        mybir.AluOpType.mult)

                # ---- phase 5: out chunk = attn(t) @ wo_shard ----
                for oc in range(NOC):
                    for tcn in range(TC // P):
                        o_ps = opsp.tile([P, TC], f32, tag="o",
                                         name=f"o_{t}_{oc}_{tcn}")
                        for hc in range(QH):
                            nc.tensor.matmul(
                                o_ps[:],
                                attnT[:, hc, tcn * P:(tcn + 1) * P],
                                wo_r[:, hc, oc * TC:(oc + 1) * TC],
                                start=(hc == 0), stop=(hc == QH - 1))
                        ob = obp.tile([P, TC], bf16, tag="ob",
                                      name=f"ob_{t}_{oc}_{tcn}")
                        if (oc + tcn) % 2 == 0:
                            nc.scalar.copy(ob[:], o_ps[:])
                        else:
                            nc.vector.tensor_copy(ob[:], o_ps[:])
                        nc.gpsimd.dma_start(
                            part[t * TC + tcn * P:t * TC + (tcn + 1) * P,
                                 oc * TC:(oc + 1) * TC], ob[:])

    nc.compile()
    return nc


def _rope_tables(positions):
    pos = positions.astype(np.float64)
    inv_freq = 1.0 / (BASE ** (np.arange(HALF, dtype=np.float64) / HALF))
    freqs = pos[:, None] * inv_freq[None, :]          # [T, 64]
    cos = np.cos(freqs)
    sin = np.sin(freqs)
    cosT = np.concatenate([cos, cos], axis=1).T       # [128, T]
    sinT = np.concatenate([-sin, sin], axis=1).T      # sign folded
    return cosT, sinT


def kernel(positions, hidden_states, wqkv, wo):
    global _COMPILED
    if _COMPILED is None:
        _COMPILED = _build()
    nc = _COMPILED

    s = 128.0 ** -0.5                                 # per-side score scale
    cosT, sinT = _rope_tables(positions)
    cosq = np.ascontiguousarray(cosT * s).astype(ml_dtypes.bfloat16)
    sinq = np.ascontiguousarray(sinT * s).astype(ml_dtypes.bfloat16)

    hidT = np.ascontiguousarray(hidden_states.T).astype(ml_dtypes.bfloat16)

    # causal staircase mask, ST layout [k, q]: one [128,128] tile serves
    # every diagonal block
    kl = np.arange(P)[:, None]
    ql = np.arange(P)[None, :]
    masks = np.where(kl <= ql, 0.0, NEG).astype(ml_dtypes.bfloat16)

    ident = np.eye(P, dtype=np.float32).astype(ml_dtypes.bfloat16)
    onesm = np.ones((P, P), dtype=np.float32).astype(ml_dtypes.bfloat16)

    in_maps = []
    for r in range(NCORES):
        qc = slice(r * QCOLS, (r + 1) * QCOLS)
        kc = slice(NH * HD + r * HD, NH * HD + (r + 1) * HD)
        vc = slice((NH + NKV) * HD + r * HD, (NH + NKV) * HD + (r + 1) * HD)
        wqkv_s = np.ascontiguousarray(
            np.concatenate([wqkv[:, qc], wqkv[:, kc], wqkv[:, vc]],
                           axis=1)).astype(ml_dtypes.bfloat16)
        wo_s = np.ascontiguousarray(wo[qc, :]).astype(ml_dtypes.bfloat16)
        in_maps.append({
            "hidT": hidT, "wqkv_s": wqkv_s, "wo_s": wo_s,
            "cosq": cosq, "sinq": sinq, "masks": masks,
            "ident": ident, "onesm": onesm,
        })

    global _LAST_IN_MAPS
    _LAST_IN_MAPS = in_maps
    res = run_bass_kernel_spmd(nc, in_maps, list(range(NCORES)))
    out = res.results[0]["part"].astype(np.float64)
    for r in range(1, NCORES):
        out += res.results[r]["part"].astype(np.float64)
    return out.astype(np.float32)


# revision 18
# speedup vs baseline: 1.6213x; 1.0095x over previous
"""InternLM3 self-attention (prefill, GQA, RoPE) on 8 Trainium2 cores.

Tensor-parallel over heads: core r owns q heads 4r..4r+3 and kv head r
(wqkv column shards, wo row shards).  Each core computes its partial
output projection; the 8 partials are summed on the host.

v3 design:
  - all matmuls bf16 (Fast Weight Load on; fp32r would serialize
    LDWEIGHTS at ~125 ns/matmul).
  - one software pipeline over the 4 token chunks: proj(t) -> rope(t)
    -> attention(g=t) -> out-proj(t); per-tile semaphores overlap the
    phases across engines.
  - softmax denominator accumulated on PE with an all-ones [128,128]
    stationary (result replicated across partitions), so 1/d is a single
    reciprocal_approx_fast on [128,512] and the normalizing multiply
    needs no broadcast at all.
  - scores/pv/denominator matmuls run 2 chunks behind the scores stream
    so PSUM bank recycling at head boundaries never stalls the PE.
  - RoPE rotate-half via SBUF->SBUF DMA partition shift; k head is
    roped first so attention never waits on it.
  - q and k both carry 128^-0.5 in the rope tables; exp() applies the
    compensating sqrt(128) via its free affine scale.
  - batched weight DMAs (descriptors fan out across all 16 queues);
    wo load is emitted late so it never delays the first projection.
"""

import numpy as np
import ml_dtypes

import concourse.bass as bass
import concourse.bacc as bacc
import concourse.mybir as mybir
import concourse.tile as tile
from concourse.bass_utils import run_bass_kernel_spmd

T = 2048
H = 4096
NH = 32
NKV = 8
HD = 128
HALF = HD // 2
BASE = 1000000.0
NCORES = 8
QH = NH // NCORES            # 4 q heads per core
QCOLS = QH * HD              # 512
SH_COLS = QCOLS + 2 * HD     # 768 wqkv cols per core
NEG = -1e30

P = 128
TC = 512                     # token chunk
NT = T // TC                 # 4
NHC = H // P                 # 32 contraction chunks
NQC = SH_COLS // P           # 6 qkv col chunks (0-3 q heads, 4 k, 5 v)
NOC = H // TC                # 8 output col chunks
LAG = 2                      # chunks the pv/denominator stream trails by

f32 = mybir.dt.float32
bf16 = mybir.dt.bfloat16

EXP_SCALE = float(np.sqrt(128.0))   # tables carry 128^-0.5 on q and k

_COMPILED = None


def _build():
    nc = bacc.Bacc("TRN2", target_bir_lowering=False, debug=False,
                   num_devices=NCORES)

    hidT = nc.dram_tensor("hidT", [H, T], bf16, kind="ExternalInput").ap()
    wqkv_s = nc.dram_tensor("wqkv_s", [H, SH_COLS], bf16,
                            kind="ExternalInput").ap()
    wo_s = nc.dram_tensor("wo_s", [QCOLS, H], bf16,
                          kind="ExternalInput").ap()
    cosq = nc.dram_tensor("cosq", [P, T], bf16, kind="ExternalInput").ap()
    sinq = nc.dram_tensor("sinq", [P, T], bf16, kind="ExternalInput").ap()
    masks = nc.dram_tensor("masks", [P, P], bf16,
                           kind="ExternalInput").ap()
    ident = nc.dram_tensor("ident", [P, P], bf16, kind="ExternalInput").ap()
    onesm = nc.dram_tensor("onesm", [P, P], bf16, kind="ExternalInput").ap()
    part = nc.dram_tensor("part", [T, H], bf16, kind="ExternalOutput").ap()

    with tile.TileContext(nc) as tc:
        with tc.tile_pool(name="keep", bufs=1) as keep, \
             tc.tile_pool(name="hid", bufs=5) as hidp, \
             tc.tile_pool(name="attn", bufs=2) as attp, \
             tc.tile_pool(name="rot", bufs=2) as rotp, \
             tc.tile_pool(name="e", bufs=6) as ep, \
             tc.tile_pool(name="rbs", bufs=2) as rbsp, \
             tc.tile_pool(name="ob", bufs=4) as obp, \
             tc.tile_pool(name="ps", bufs=4, space="PSUM") as psp, \
             tc.tile_pool(name="acc", bufs=1, space="PSUM") as accp, \
             tc.tile_pool(name="ops", bufs=2, space="PSUM") as opsp:

            # ---- long-lived SBUF ----
            wq = keep.tile([P, NHC, SH_COLS], bf16)        # 48 KB/part
            wo_r = keep.tile([P, QH, H], bf16)             # 32 KB/part
            qkvT = keep.tile([P, NQC, T], bf16)            # 24 KB/part
            ct = keep.tile([P, T], bf16, tag="cosq_t")     # 4 KB
            st_t = keep.tile([P, T], bf16, tag="sinq_t")   # 4 KB
            mt = keep.tile([P, P], bf16, tag="masks_t")    # staircase
            vnat = keep.tile([P, T // P, P], bf16, tag="vnat")  # 4 KB
            idt = keep.tile([P, P], bf16, tag="ident_t")
            o_m = keep.tile([P, P], bf16, tag="onesm_t")

            # hid(0) + wq first: these gate the very first matmuls.
            QTR = NHC // 4
            hid_t0 = []
            for qi in range(4):
                hq = hidp.tile([P, QTR, TC], bf16, tag="hid",
                               name=f"hid_0_{qi}")
                nc.sync.dma_start(
                    hq[:],
                    hidT[qi * QTR * P:(qi + 1) * QTR * P, 0:TC].rearrange(
                        "(h p) n -> p h n", p=P))
                hid_t0.append(hq)
            for c in (4, 5, 0, 1, 2, 3):      # proj column order
                nc.sync.dma_start(
                    wq[:, :, c * P:(c + 1) * P],
                    wqkv_s[:, c * P:(c + 1) * P].rearrange(
                        "(h p) c -> p h c", p=P))
            nc.scalar.dma_start(ct[:], cosq[:])
            nc.scalar.dma_start(st_t[:], sinq[:])
            nc.scalar.dma_start(mt[:], masks[:])
            nc.scalar.dma_start(idt[:], ident[:])
            nc.scalar.dma_start(o_m[:], onesm[:])

            for t in range(NT):
                tsl = slice(t * TC, (t + 1) * TC)

                if t == 0:
                    hid_q = hid_t0
                else:
                    hid_q = []
                    for qi in range(4):
                        hq = hidp.tile([P, QTR, TC], bf16, tag="hid",
                                       name=f"hid_{t}_{qi}")
                        nc.sync.dma_start(
                            hq[:],
                            hidT[qi * QTR * P:(qi + 1) * QTR * P,
                                 tsl].rearrange("(h p) n -> p h n", p=P))
                        hid_q.append(hq)

                # ---- phase 1: qkv^T chunk = wqkv^T @ hidden^T ----
                # k and v first so rope(k) / v-transpose overlap the q cols.
                def proj_col(c):
                    qps = psp.tile([P, TC], f32, tag="ps",
                                   name=f"qps_{t}_{c}")
                    for h in range(NHC):
                        nc.tensor.matmul(
                            qps[:], wq[:, h, c * P:(c + 1) * P],
                            hid_q[h // QTR][:, h % QTR, :],
                            start=(h == 0), stop=(h == NHC - 1))
                    nc.scalar.copy(qkvT[:, c, tsl], qps[:])

                def rope_col(idx):
                    x = qkvT[:, idx, tsl]
                    rot = rotp.tile([P, TC], bf16, tag="rot",
                                    name=f"rot_{t}_{idx}")
                    nc.gpsimd.dma_start(rot[0:HALF, :],
                                        qkvT[HALF:P, idx, tsl])
                    nc.gpsimd.dma_start(rot[HALF:P, :],
                                        qkvT[0:HALF, idx, tsl])
                    nc.vector.tensor_tensor(
                        rot[:], rot[:], st_t[:, tsl], mybir.AluOpType.mult)
                    nc.vector.tensor_tensor(
                        x, x, ct[:, tsl], mybir.AluOpType.mult)
                    nc.vector.tensor_tensor(
                        x, x, rot[:], mybir.AluOpType.add)

                proj_col(4)                      # k
                proj_col(5)                      # v
                rope_col(QH)                     # rope k immediately
                for j in range(TC // P):         # v transpose
                    kc = t * (TC // P) + j
                    tp = psp.tile([P, TC], bf16, tag="ps", name=f"vt_{kc}")
                    nc.tensor.transpose(
                        tp[:, 0:P], qkvT[:, 5, kc * P:(kc + 1) * P], idt[:])
                    nc.scalar.copy(vnat[:, kc, :], tp[:, 0:P])
                for c in range(QH):              # q heads + their rope
                    proj_col(c)
                    rope_col(c)

                if t == 0:
                    # wo load, deferred so it never races the hot path
                    for hc in range(QH):
                        nc.sync.dma_start(wo_r[:, hc, :],
                                          wo_s[hc * P:(hc + 1) * P, :])

                # ---- phase 4: causal attention, q group g == t ----
                attnT = attp.tile([P, QH, TC], bf16, tag="attnT",
                                  name=f"attnT_{t}")
                kmax = 4 * (t + 1)
                for head in range(QH):
                    d_rep = accp.tile([P, TC], f32, tag="d",
                                      name=f"d_{t}_{head}")
                    pv = accp.tile([P, TC], f32, tag="pv",
                                   name=f"pv_{t}_{head}")
                    es = []

                    def drain_one():
                        pkc, pe = es.pop(0)
                        nc.tensor.matmul(d_rep[:], o_m[:], pe[:],
                                         start=(pkc == 0),
                                         stop=(pkc == kmax - 1))
                        nc.tensor.matmul(pv[:], vnat[:, pkc, :], pe[:],
                                         start=(pkc == 0),
                                         stop=(pkc == kmax - 1))

                    for kc in range(kmax):
                        st_ps = psp.tile([P, TC], f32, tag="ps",
                                         name=f"st_{t}_{head}_{kc}")
                        nc.tensor.matmul(
                            st_ps[:],
                            qkvT[:, QH, kc * P:(kc + 1) * P],
                            qkvT[:, head, tsl],
                            start=True, stop=True)
                        if len(es) >= LAG:
                            drain_one()
                        j = kc - 4 * t
                        e = ep.tile([P, TC], bf16, tag="E",
                                    name=f"e_{t}_{head}_{kc}")
                        if j >= 0:
                            # diagonal block: cols < 128j fully masked,
                            # staircase only in cols [128j, 128j+128)
                            nc.vector.tensor_tensor(
                                st_ps[:, j * P:(j + 1) * P],
                                st_ps[:, j * P:(j + 1) * P], mt[:],
                                mybir.AluOpType.add)
                            if j > 0:
                                nc.gpsimd.memset(e[:, 0:j * P], 0)
                            nc.scalar.activation(
                                e[:, j * P:], st_ps[:, j * P:],
                                mybir.ActivationFunctionType.Exp,
                                scale=EXP_SCALE)
                        else:
                            nc.scalar.activation(
                                e[:], st_ps[:],
                                mybir.ActivationFunctionType.Exp,
                                scale=EXP_SCALE)
                        es.append((kc, e))
                    while es:
                        drain_one()
                    rbs = rbsp.tile([P, TC], f32, tag="rbs",
                                    name=f"rbs_{t}_{head}")
                    nc.vector.reciprocal_approx_fast(rbs[:], d_rep[:])
                    nc.vector.tensor_tensor(
                        attnT[:, head, :], pv[:], rbs[:],
                        mybir.AluOpType.mult)

                # ---- phase 5: out chunk = attn(t) @ wo_shard ----
                for oc in range(NOC):
                    for tcn in range(TC // P):
                        o_ps = opsp.tile([P, TC], f32, tag="o",
                                         name=f"o_{t}_{oc}_{tcn}")
                        for hc in range(QH):
                            nc.tensor.matmul(
                                o_ps[:],
                                attnT[:, hc, tcn * P:(tcn + 1) * P],
                                wo_r[:, hc, oc * TC:(oc + 1) * TC],
                                start=(hc == 0), stop=(hc == QH - 1))
                        ob = obp.tile([P, TC], bf16, tag="ob",
                                      name=f"ob_{t}_{oc}_{tcn}")
                        nc.scalar.copy(ob[:], o_ps[:])
                        nc.gpsimd.dma_start(
                            part[t * TC + tcn * P:t * TC + (tcn + 1) * P,
                                 oc * TC:(oc + 1) * TC], ob[:])

    nc.compile()
    return nc


def _rope_tables(positions):
    pos = positions.astype(np.float64)
    inv_freq = 1.0 / (BASE ** (np.arange(HALF, dtype=np.float64) / HALF))
    freqs = pos[:, None] * inv_freq[None, :]          # [T, 64]
    cos = np.cos(freqs)
    sin = np.sin(freqs)
    cosT = np.concatenate([cos, cos], axis=1).T       # [128, T]
    sinT = np.concatenate([-sin, sin], axis=1).T      # sign folded
    return cosT, sinT


def kernel(positions, hidden_states, wqkv, wo):
    global _COMPILED
    if _COMPILED is None:
        _COMPILED = _build()
    nc = _COMPILED

    s = 128.0 ** -0.5                                 # per-side score scale
    cosT, sinT = _rope_tables(positions)
    cosq = np.ascontiguousarray(cosT * s).astype(ml_dtypes.bfloat16)
    sinq = np.ascontiguousarray(sinT * s).astype(ml_dtypes.bfloat16)

    hidT = np.ascontiguousarray(hidden_states.T).astype(ml_dtypes.bfloat16)

    # causal staircase mask, ST layout [k, q]: one [128,128] tile serves
    # every diagonal block
    kl = np.arange(P)[:, None]
    ql = np.arange(P)[None, :]
    masks = np.where(kl <= ql, 0.0, NEG).astype(ml_dtypes.bfloat16)

    ident = np.eye(P, dtype=np.float32).astype(ml_dtypes.bfloat16)
    onesm = np.ones((P, P), dtype=np.float32).astype(ml_dtypes.bfloat16)

    in_maps = []
    for r in range(NCORES):
        qc = slice(r * QCOLS, (r + 1) * QCOLS)
        kc = slice(NH * HD + r * HD, NH * HD + (r + 1) * HD)
        vc = slice((NH + NKV) * HD + r * HD, (NH + NKV) * HD + (r + 1) * HD)
        wqkv_s = np.ascontiguousarray(
            np.concatenate([wqkv[:, qc], wqkv[:, kc], wqkv[:, vc]],
                           axis=1)).astype(ml_dtypes.bfloat16)
        wo_s = np.ascontiguousarray(wo[qc, :]).astype(ml_dtypes.bfloat16)
        in_maps.append({
            "hidT": hidT, "wqkv_s": wqkv_s, "wo_s": wo_s,
            "cosq": cosq, "sinq": sinq, "masks": masks,
            "ident": ident, "onesm": onesm,
        })

    global _LAST_IN_MAPS
    _LAST_IN_MAPS = in_maps
    res = run_bass_kernel_spmd(nc, in_maps, list(range(NCORES)))
    out = res.results[0]["part"].astype(np.float64)
    for r in range(1, NCORES):
        out += res.results[r]["part"].astype(np.float64)
    return out.astype(np.float32)


# revision 25
# speedup vs baseline: 1.6333x; 1.0074x over previous
"""InternLM3 self-attention (prefill, GQA, RoPE) on 8 Trainium2 cores.

Tensor-parallel over heads: core r owns q heads 4r..4r+3 and kv head r
(wqkv column shards, wo row shards).  Each core computes its partial
output projection; the 8 partials are summed on the host.

v3 design:
  - all matmuls bf16 (Fast Weight Load on; fp32r would serialize
    LDWEIGHTS at ~125 ns/matmul).
  - one software pipeline over the 4 token chunks: proj(t) -> rope(t)
    -> attention(g=t) -> out-proj(t); per-tile semaphores overlap the
    phases across engines.
  - softmax denominator accumulated on PE with an all-ones [128,128]
    stationary (result replicated across partitions), so 1/d is a single
    reciprocal_approx_fast on [128,512] and the normalizing multiply
    needs no broadcast at all.
  - scores/pv/denominator matmuls run 2 chunks behind the scores stream
    so PSUM bank recycling at head boundaries never stalls the PE.
  - RoPE rotate-half via SBUF->SBUF DMA partition shift; k head is
    roped first so attention never waits on it.
  - q and k both carry 128^-0.5 in the rope tables; exp() applies the
    compensating sqrt(128) via its free affine scale.
  - batched weight DMAs (descriptors fan out across all 16 queues);
    wo load is emitted late so it never delays the first projection.
"""

import numpy as np
import ml_dtypes

import concourse.bass as bass
import concourse.bacc as bacc
import concourse.mybir as mybir
import concourse.tile as tile
from concourse.bass_utils import run_bass_kernel_spmd

T = 2048
H = 4096
NH = 32
NKV = 8
HD = 128
HALF = HD // 2
BASE = 1000000.0
NCORES = 8
QH = NH // NCORES            # 4 q heads per core
QCOLS = QH * HD              # 512
SH_COLS = QCOLS + 2 * HD     # 768 wqkv cols per core
NEG = -1e30

P = 128
TC = 512                     # token chunk
NT = T // TC                 # 4
NHC = H // P                 # 32 contraction chunks
NQC = SH_COLS // P           # 6 qkv col chunks (0-3 q heads, 4 k, 5 v)
NOC = H // TC                # 8 output col chunks
LAG = 2                      # chunks the pv/denominator stream trails by

f32 = mybir.dt.float32
bf16 = mybir.dt.bfloat16

EXP_SCALE = float(np.sqrt(128.0))   # tables carry 128^-0.5 on q and k

_COMPILED = None


def _build():
    nc = bacc.Bacc("TRN2", target_bir_lowering=False, debug=False,
                   num_devices=NCORES)

    hidT = nc.dram_tensor("hidT", [H, T], bf16, kind="ExternalInput").ap()
    wqkv_s = nc.dram_tensor("wqkv_s", [H, SH_COLS], bf16,
                            kind="ExternalInput").ap()
    wo_s = nc.dram_tensor("wo_s", [QCOLS, H], bf16,
                          kind="ExternalInput").ap()
    cosq = nc.dram_tensor("cosq", [P, T], bf16, kind="ExternalInput").ap()
    sinq = nc.dram_tensor("sinq", [P, T], bf16, kind="ExternalInput").ap()
    masks = nc.dram_tensor("masks", [P, P], bf16,
                           kind="ExternalInput").ap()
    ident = nc.dram_tensor("ident", [P, P], bf16, kind="ExternalInput").ap()
    onesm = nc.dram_tensor("onesm", [P, P], bf16, kind="ExternalInput").ap()
    rperm = nc.dram_tensor("rperm", [P, P], bf16, kind="ExternalInput").ap()
    part = nc.dram_tensor("part", [T, H], bf16, kind="ExternalOutput").ap()

    with tile.TileContext(nc) as tc:
        with tc.tile_pool(name="keep", bufs=1) as keep, \
             tc.tile_pool(name="hid", bufs=10) as hidp, \
             tc.tile_pool(name="attn", bufs=2) as attp, \
             tc.tile_pool(name="rot", bufs=2) as rotp, \
             tc.tile_pool(name="e", bufs=6) as ep, \
             tc.tile_pool(name="rbs", bufs=2) as rbsp, \
             tc.tile_pool(name="ob", bufs=6) as obp, \
             tc.tile_pool(name="ps", bufs=4, space="PSUM") as psp, \
             tc.tile_pool(name="acc", bufs=1, space="PSUM") as accp, \
             tc.tile_pool(name="ops", bufs=2, space="PSUM") as opsp:

            # ---- long-lived SBUF ----
            wq = keep.tile([P, NHC, SH_COLS], bf16)        # 48 KB/part
            wo_r = keep.tile([P, QH, H], bf16)             # 32 KB/part
            qkvT = keep.tile([P, NQC, T], bf16)            # 24 KB/part
            ct = keep.tile([P, T], bf16, tag="cosq_t")     # 4 KB
            st_t = keep.tile([P, T], bf16, tag="sinq_t")   # 4 KB
            mt = keep.tile([P, P], bf16, tag="masks_t")    # staircase
            vnat = keep.tile([P, T // P, P], bf16, tag="vnat")  # 4 KB
            idt = keep.tile([P, P], bf16, tag="ident_t")
            o_m = keep.tile([P, P], bf16, tag="onesm_t")
            rp = keep.tile([P, P], bf16, tag="rperm_t")

            # hid(0) + wq first: these gate the very first matmuls.
            QTR = NHC // 8
            hid_t0 = []
            for qi in range(8):
                hq = hidp.tile([P, QTR, TC], bf16, tag="hid",
                               name=f"hid_0_{qi}")
                nc.sync.dma_start(
                    hq[:],
                    hidT[qi * QTR * P:(qi + 1) * QTR * P, 0:TC].rearrange(
                        "(h p) n -> p h n", p=P))
                hid_t0.append(hq)
            for c in (4, 5, 0, 1, 2, 3):      # proj column order
                nc.sync.dma_start(
                    wq[:, :, c * P:(c + 1) * P],
                    wqkv_s[:, c * P:(c + 1) * P].rearrange(
                        "(h p) c -> p h c", p=P))
            nc.scalar.dma_start(ct[:], cosq[:])
            nc.scalar.dma_start(st_t[:], sinq[:])
            nc.scalar.dma_start(mt[:], masks[:])
            nc.scalar.dma_start(idt[:], ident[:])
            nc.scalar.dma_start(o_m[:], onesm[:])
            nc.scalar.dma_start(rp[:], rperm[:])

            for t in range(NT):
                tsl = slice(t * TC, (t + 1) * TC)

                if t == 0:
                    hid_q = hid_t0
                else:
                    hid_q = []
                    for qi in range(8):
                        hq = hidp.tile([P, QTR, TC], bf16, tag="hid",
                                       name=f"hid_{t}_{qi}")
                        nc.sync.dma_start(
                            hq[:],
                            hidT[qi * QTR * P:(qi + 1) * QTR * P,
                                 tsl].rearrange("(h p) n -> p h n", p=P))
                        hid_q.append(hq)

                # ---- phase 1: qkv^T chunk = wqkv^T @ hidden^T ----
                # k and v first so rope(k) / v-transpose overlap the q cols.
                def proj_col(c):
                    qps = psp.tile([P, TC], f32, tag="ps",
                                   name=f"qps_{t}_{c}")
                    for h in range(NHC):
                        nc.tensor.matmul(
                            qps[:], wq[:, h, c * P:(c + 1) * P],
                            hid_q[h // QTR][:, h % QTR, :],
                            start=(h == 0), stop=(h == NHC - 1))
                    nc.scalar.copy(qkvT[:, c, tsl], qps[:])

                def rope_col(idx):
                    x = qkvT[:, idx, tsl]
                    rot_ps = psp.tile([P, TC], f32, tag="ps",
                                      name=f"rotp_{t}_{idx}")
                    nc.tensor.matmul(rot_ps[:], rp[:], x,
                                     start=True, stop=True)
                    rot = rotp.tile([P, TC], bf16, tag="rot",
                                    name=f"rot_{t}_{idx}")
                    nc.vector.tensor_tensor(
                        rot[:], rot_ps[:], st_t[:, tsl],
                        mybir.AluOpType.mult)
                    nc.vector.tensor_tensor(
                        x, x, ct[:, tsl], mybir.AluOpType.mult)
                    nc.vector.tensor_tensor(
                        x, x, rot[:], mybir.AluOpType.add)

                proj_col(4)                      # k
                proj_col(5)                      # v
                rope_col(QH)                     # rope k immediately
                for j in range(TC // P):         # v transpose
                    kc = t * (TC // P) + j
                    tp = psp.tile([P, TC], bf16, tag="ps", name=f"vt_{kc}")
                    nc.tensor.transpose(
                        tp[:, 0:P], qkvT[:, 5, kc * P:(kc + 1) * P], idt[:])
                    nc.scalar.copy(vnat[:, kc, :], tp[:, 0:P])
                for c in range(QH):              # q heads + their rope
                    proj_col(c)
                    rope_col(c)

                if t == 0:
                    # wo load, deferred so it never races the hot path
                    for hc in range(QH):
                        nc.sync.dma_start(wo_r[:, hc, :],
                                          wo_s[hc * P:(hc + 1) * P, :])

                # ---- phase 4: causal attention, q group g == t ----
                attnT = attp.tile([P, QH, TC], bf16, tag="attnT",
                                  name=f"attnT_{t}")
                kmax = 4 * (t + 1)
                for head in range(QH):
                    d_rep = accp.tile([P, TC], f32, tag="d",
                                      name=f"d_{t}_{head}")
                    pv = accp.tile([P, TC], f32, tag="pv",
                                   name=f"pv_{t}_{head}")
                    es = []

                    def drain_one():
                        pkc, pe = es.pop(0)
                        nc.tensor.matmul(d_rep[:], o_m[:], pe[:],
                                         start=(pkc == 0),
                                         stop=(pkc == kmax - 1))
                        nc.tensor.matmul(pv[:], vnat[:, pkc, :], pe[:],
                                         start=(pkc == 0),
                                         stop=(pkc == kmax - 1))

                    for kc in range(kmax):
                        st_ps = psp.tile([P, TC], f32, tag="ps",
                                         name=f"st_{t}_{head}_{kc}")
                        nc.tensor.matmul(
                            st_ps[:],
                            qkvT[:, QH, kc * P:(kc + 1) * P],
                            qkvT[:, head, tsl],
                            start=True, stop=True)
                        if len(es) >= LAG:
                            drain_one()
                        j = kc - 4 * t
                        e = ep.tile([P, TC], bf16, tag="E",
                                    name=f"e_{t}_{head}_{kc}")
                        if j >= 0:
                            # diagonal block: cols < 128j fully masked,
                            # staircase only in cols [128j, 128j+128)
                            nc.vector.tensor_tensor(
                                st_ps[:, j * P:(j + 1) * P],
                                st_ps[:, j * P:(j + 1) * P], mt[:],
                                mybir.AluOpType.add)
                            if j > 0:
                                nc.gpsimd.memset(e[:, 0:j * P], 0)
                            nc.scalar.activation(
                                e[:, j * P:], st_ps[:, j * P:],
                                mybir.ActivationFunctionType.Exp,
                                scale=EXP_SCALE)
                        else:
                            nc.scalar.activation(
                                e[:], st_ps[:],
                                mybir.ActivationFunctionType.Exp,
                                scale=EXP_SCALE)
                        es.append((kc, e))
                    while es:
                        drain_one()
                    rbs = rbsp.tile([P, TC], f32, tag="rbs",
                                    name=f"rbs_{t}_{head}")
                    nc.vector.reciprocal_approx_fast(rbs[:], d_rep[:])
                    nc.vector.tensor_tensor(
                        attnT[:, head, :], pv[:], rbs[:],
                        mybir.AluOpType.mult)

                # ---- phase 5: out chunk = attn(t) @ wo_shard ----
                for oc in range(NOC):
                    for tcn in range(TC // P):
                        o_ps = opsp.tile([P, TC], f32, tag="o",
                                         name=f"o_{t}_{oc}_{tcn}")
                        for hc in range(QH):
                            nc.tensor.matmul(
                                o_ps[:],
                                attnT[:, hc, tcn * P:(tcn + 1) * P],
                                wo_r[:, hc, oc * TC:(oc + 1) * TC],
                                start=(hc == 0), stop=(hc == QH - 1))
                        ob = obp.tile([P, TC], bf16, tag="ob",
                                      name=f"ob_{t}_{oc}_{tcn}")
                        nc.scalar.copy(ob[:], o_ps[:])
                        nc.gpsimd.dma_start(
                            part[t * TC + tcn * P:t * TC + (tcn + 1) * P,
                                 oc * TC:(oc + 1) * TC], ob[:])

    nc.compile()
    return nc


def _rope_tables(positions):
    pos = positions.astype(np.float64)
    inv_freq = 1.0 / (BASE ** (np.arange(HALF, dtype=np.float64) / HALF))
    freqs = pos[:, None] * inv_freq[None, :]          # [T, 64]
    cos = np.cos(freqs)
    sin = np.sin(freqs)
    cosT = np.concatenate([cos, cos], axis=1).T       # [128, T]
    sinT = np.concatenate([-sin, sin], axis=1).T      # sign folded
    return cosT, sinT


def kernel(positions, hidden_states, wqkv, wo):
    global _COMPILED
    if _COMPILED is None:
        _COMPILED = _build()
    nc = _COMPILED

    s = 128.0 ** -0.5                                 # per-side score scale
    cosT, sinT = _rope_tables(positions)
    cosq = np.ascontiguousarray(cosT * s).astype(ml_dtypes.bfloat16)
    sinq = np.ascontiguousarray(sinT * s).astype(ml_dtypes.bfloat16)

    hidT = np.ascontiguousarray(hidden_states.T).astype(ml_dtypes.bfloat16)

    # causal staircase mask, ST layout [k, q]: one [128,128] tile serves
    # every diagonal block
    kl = np.arange(P)[:, None]
    ql = np.arange(P)[None, :]
    masks = np.where(kl <= ql, 0.0, NEG).astype(ml_dtypes.bfloat16)

    ident = np.eye(P, dtype=np.float32).astype(ml_dtypes.bfloat16)
    onesm = np.ones((P, P), dtype=np.float32).astype(ml_dtypes.bfloat16)
    rperm_np = np.zeros((P, P), dtype=np.float32)
    for m in range(P):
        rperm_np[(m + HALF) % P, m] = 1.0             # out[m]=x[(m+64)%128]
    rperm_np = rperm_np.astype(ml_dtypes.bfloat16)

    in_maps = []
    for r in range(NCORES):
        qc = slice(r * QCOLS, (r + 1) * QCOLS)
        kc = slice(NH * HD + r * HD, NH * HD + (r + 1) * HD)
        vc = slice((NH + NKV) * HD + r * HD, (NH + NKV) * HD + (r + 1) * HD)
        wqkv_s = np.ascontiguousarray(
            np.concatenate([wqkv[:, qc], wqkv[:, kc], wqkv[:, vc]],
                           axis=1)).astype(ml_dtypes.bfloat16)
        wo_s = np.ascontiguousarray(wo[qc, :]).astype(ml_dtypes.bfloat16)
        in_maps.append({
            "hidT": hidT, "wqkv_s": wqkv_s, "wo_s": wo_s,
            "cosq": cosq, "sinq": sinq, "masks": masks,
            "ident": ident, "onesm": onesm, "rperm": rperm_np,
        })

    global _LAST_IN_MAPS
    _LAST_IN_MAPS = in_maps
    res = run_bass_kernel_spmd(nc, in_maps, list(range(NCORES)))
    out = res.results[0]["part"].astype(np.float64)
    for r in range(1, NCORES):
        out += res.results[r]["part"].astype(np.float64)
    return out.astype(np.float32)


# revision 28
# speedup vs baseline: 1.7081x; 1.0458x over previous
"""InternLM3 self-attention (prefill, GQA, RoPE) on 8 Trainium2 cores.

Tensor-parallel over heads: core r owns q heads 4r..4r+3 and kv head r
(wqkv column shards, wo row shards).  Each core computes its partial
output projection; the 8 partials are summed on the host.

v3 design:
  - all matmuls bf16 (Fast Weight Load on; fp32r would serialize
    LDWEIGHTS at ~125 ns/matmul).
  - one software pipeline over the 4 token chunks: proj(t) -> rope(t)
    -> attention(g=t) -> out-proj(t); per-tile semaphores overlap the
    phases across engines.
  - softmax denominator accumulated on PE with an all-ones [128,128]
    stationary (result replicated across partitions), so 1/d is a single
    reciprocal_approx_fast on [128,512] and the normalizing multiply
    needs no broadcast at all.
  - scores/pv/denominator matmuls run 2 chunks behind the scores stream
    so PSUM bank recycling at head boundaries never stalls the PE.
  - RoPE rotate-half via SBUF->SBUF DMA partition shift; k head is
    roped first so attention never waits on it.
  - q and k both carry 128^-0.5 in the rope tables; exp() applies the
    compensating sqrt(128) via its free affine scale.
  - batched weight DMAs (descriptors fan out across all 16 queues);
    wo load is emitted late so it never delays the first projection.
"""

import numpy as np
import ml_dtypes

import concourse.bass as bass
import concourse.bacc as bacc
import concourse.mybir as mybir
import concourse.tile as tile
from concourse.bass_utils import run_bass_kernel_spmd

T = 2048
H = 4096
NH = 32
NKV = 8
HD = 128
HALF = HD // 2
BASE = 1000000.0
NCORES = 8
QH = NH // NCORES            # 4 q heads per core
QCOLS = QH * HD              # 512
SH_COLS = QCOLS + 2 * HD     # 768 wqkv cols per core
NEG = -1e30

P = 128
TC = 512                     # token chunk
NT = T // TC                 # 4
NHC = H // P                 # 32 contraction chunks
NQC = SH_COLS // P           # 6 qkv col chunks (0-3 q heads, 4 k, 5 v)
NOC = H // TC                # 8 output col chunks
LAG = 2                      # chunks the pv/denominator stream trails by

f32 = mybir.dt.float32
bf16 = mybir.dt.bfloat16

EXP_SCALE = float(np.sqrt(128.0))   # tables carry 128^-0.5 on q and k

_COMPILED = None


def _build():
    nc = bacc.Bacc("TRN2", target_bir_lowering=False, debug=False,
                   num_devices=NCORES)

    hidT = nc.dram_tensor("hidT", [H, T], bf16, kind="ExternalInput").ap()
    wqkv_s = nc.dram_tensor("wqkv_s", [H, SH_COLS], bf16,
                            kind="ExternalInput").ap()
    wo_s = nc.dram_tensor("wo_s", [QCOLS, H], bf16,
                          kind="ExternalInput").ap()
    cosq = nc.dram_tensor("cosq", [P, T], bf16, kind="ExternalInput").ap()
    sinq = nc.dram_tensor("sinq", [P, T], bf16, kind="ExternalInput").ap()
    masks = nc.dram_tensor("masks", [P, P], bf16,
                           kind="ExternalInput").ap()
    ident = nc.dram_tensor("ident", [P, P], bf16, kind="ExternalInput").ap()
    onesm = nc.dram_tensor("onesm", [P, P], bf16, kind="ExternalInput").ap()
    rperm = nc.dram_tensor("rperm", [P, P], bf16, kind="ExternalInput").ap()
    part = nc.dram_tensor("part", [T, H], bf16, kind="ExternalOutput").ap()

    with tile.TileContext(nc) as tc:
        with tc.tile_pool(name="keep", bufs=1) as keep, \
             tc.tile_pool(name="hid", bufs=10) as hidp, \
             tc.tile_pool(name="attn", bufs=2) as attp, \
             tc.tile_pool(name="rot", bufs=2) as rotp, \
             tc.tile_pool(name="e", bufs=6) as ep, \
             tc.tile_pool(name="rbs", bufs=2) as rbsp, \
             tc.tile_pool(name="ob", bufs=6) as obp, \
             tc.tile_pool(name="ps", bufs=4, space="PSUM") as psp, \
             tc.tile_pool(name="acc", bufs=1, space="PSUM") as accp, \
             tc.tile_pool(name="ops", bufs=2, space="PSUM") as opsp:

            # ---- long-lived SBUF ----
            wq = keep.tile([P, NHC, SH_COLS], bf16)        # 48 KB/part
            wo_r = keep.tile([P, QH, H], bf16)             # 32 KB/part
            qkvT = keep.tile([P, NQC, T], bf16)            # 24 KB/part
            ct = keep.tile([P, T], bf16, tag="cosq_t")     # 4 KB
            st_t = keep.tile([P, T], bf16, tag="sinq_t")   # 4 KB
            mt = keep.tile([P, P], bf16, tag="masks_t")    # staircase
            vnat = keep.tile([P, T // P, P], bf16, tag="vnat")  # 4 KB
            idt = keep.tile([P, P], bf16, tag="ident_t")
            o_m = keep.tile([P, P], bf16, tag="onesm_t")
            rp = keep.tile([P, P], bf16, tag="rperm_t")

            # hid(0) + wq interleaved in first-needed order: the DMA queues
            # are FIFO, so issue order decides who gets early bandwidth.
            QTR = NHC // 8
            hid_t0 = [hidp.tile([P, QTR, TC], bf16, tag="hid",
                                name=f"hid_0_{qi}") for qi in range(8)]

            def hid_dma(hq, qi, t):
                nc.sync.dma_start(
                    hq[:],
                    hidT[qi * QTR * P:(qi + 1) * QTR * P,
                         t * TC:(t + 1) * TC].rearrange(
                        "(h p) n -> p h n", p=P))

            def wq_dma(c, half):
                hh = half * (NHC // 2)
                nc.sync.dma_start(
                    wq[:, hh:hh + NHC // 2, c * P:(c + 1) * P],
                    wqkv_s[hh * P:hh * P + NHC // 2 * P,
                           c * P:(c + 1) * P].rearrange(
                        "(h p) c -> p h c", p=P))

            hid_dma(hid_t0[0], 0, 0)
            wq_dma(4, 0)
            hid_dma(hid_t0[1], 1, 0)
            hid_dma(hid_t0[2], 2, 0)
            hid_dma(hid_t0[3], 3, 0)
            wq_dma(4, 1)
            for qi in range(4, 8):
                hid_dma(hid_t0[qi], qi, 0)
            for c in (5, 0, 1, 2, 3):
                wq_dma(c, 0)
                wq_dma(c, 1)
            nc.scalar.dma_start(ct[:], cosq[:])
            nc.scalar.dma_start(st_t[:], sinq[:])
            nc.scalar.dma_start(mt[:], masks[:])
            nc.scalar.dma_start(idt[:], ident[:])
            nc.scalar.dma_start(o_m[:], onesm[:])
            nc.scalar.dma_start(rp[:], rperm[:])

            for t in range(NT):
                tsl = slice(t * TC, (t + 1) * TC)

                if t == 0:
                    hid_q = hid_t0
                else:
                    hid_q = []
                    for qi in range(8):
                        hq = hidp.tile([P, QTR, TC], bf16, tag="hid",
                                       name=f"hid_{t}_{qi}")
                        hid_dma(hq, qi, t)
                        hid_q.append(hq)

                # ---- phase 1: qkv^T chunk = wqkv^T @ hidden^T ----
                # k and v first so rope(k) / v-transpose overlap the q cols.
                def proj_col(c):
                    qps = psp.tile([P, TC], f32, tag="ps",
                                   name=f"qps_{t}_{c}")
                    for h in range(NHC):
                        nc.tensor.matmul(
                            qps[:], wq[:, h, c * P:(c + 1) * P],
                            hid_q[h // QTR][:, h % QTR, :],
                            start=(h == 0), stop=(h == NHC - 1))
                    nc.scalar.copy(qkvT[:, c, tsl], qps[:])

                def rope_col(idx):
                    x = qkvT[:, idx, tsl]
                    rot_ps = psp.tile([P, TC], f32, tag="ps",
                                      name=f"rotp_{t}_{idx}")
                    nc.tensor.matmul(rot_ps[:], rp[:], x,
                                     start=True, stop=True)
                    rot = rotp.tile([P, TC], bf16, tag="rot",
                                    name=f"rot_{t}_{idx}")
                    nc.vector.tensor_tensor(
                        rot[:], rot_ps[:], st_t[:, tsl],
                        mybir.AluOpType.mult)
                    nc.vector.tensor_tensor(
                        x, x, ct[:, tsl], mybir.AluOpType.mult)
                    nc.vector.tensor_tensor(
                        x, x, rot[:], mybir.AluOpType.add)

                # rope/vt are emitted one c-group late so the PSUM->SBUF
                # copy they read never stalls the in-order PE stream.
                proj_col(4)                      # k
                proj_col(5)                      # v
                rope_col(QH)                     # k rope (c4 copy now done)
                proj_col(0)
                for j in range(TC // P):         # v transpose
                    kc = t * (TC // P) + j
                    tp = psp.tile([P, TC], bf16, tag="ps", name=f"vt_{kc}")
                    nc.tensor.transpose(
                        tp[:, 0:P], qkvT[:, 5, kc * P:(kc + 1) * P], idt[:])
                    nc.scalar.copy(vnat[:, kc, :], tp[:, 0:P])
                proj_col(1)
                rope_col(0)
                proj_col(2)
                rope_col(1)
                proj_col(3)
                rope_col(2)

                if t == 0:
                    # wo load, deferred so it never races the hot path
                    for hc in range(QH):
                        nc.sync.dma_start(wo_r[:, hc, :],
                                          wo_s[hc * P:(hc + 1) * P, :])

                # ---- phase 4: causal attention, q group g == t ----
                attnT = attp.tile([P, QH, TC], bf16, tag="attnT",
                                  name=f"attnT_{t}")
                kmax = 4 * (t + 1)
                for head in range(QH):
                    if head == 1:
                        rope_col(3)              # q3 rope hidden under head 0
                    d_rep = accp.tile([P, TC], f32, tag="d",
                                      name=f"d_{t}_{head}")
                    pv = accp.tile([P, TC], f32, tag="pv",
                                   name=f"pv_{t}_{head}")
                    es = []

                    def drain_one():
                        pkc, pe = es.pop(0)
                        nc.tensor.matmul(d_rep[:], o_m[:], pe[:],
                                         start=(pkc == 0),
                                         stop=(pkc == kmax - 1))
                        nc.tensor.matmul(pv[:], vnat[:, pkc, :], pe[:],
                                         start=(pkc == 0),
                                         stop=(pkc == kmax - 1))

                    for kc in range(kmax):
                        st_ps = psp.tile([P, TC], f32, tag="ps",
                                         name=f"st_{t}_{head}_{kc}")
                        nc.tensor.matmul(
                            st_ps[:],
                            qkvT[:, QH, kc * P:(kc + 1) * P],
                            qkvT[:, head, tsl],
                            start=True, stop=True)
                        if len(es) >= LAG:
                            drain_one()
                        j = kc - 4 * t
                        e = ep.tile([P, TC], bf16, tag="E",
                                    name=f"e_{t}_{head}_{kc}")
                        if j >= 0:
                            # diagonal block: cols < 128j fully masked,
                            # staircase only in cols [128j, 128j+128)
                            nc.vector.tensor_tensor(
                                st_ps[:, j * P:(j + 1) * P],
                                st_ps[:, j * P:(j + 1) * P], mt[:],
                                mybir.AluOpType.add)
                            if j > 0:
                                nc.gpsimd.memset(e[:, 0:j * P], 0)
                            nc.scalar.activation(
                                e[:, j * P:], st_ps[:, j * P:],
                                mybir.ActivationFunctionType.Exp,
                                scale=EXP_SCALE)
                        else:
                            nc.scalar.activation(
                                e[:], st_ps[:],
                                mybir.ActivationFunctionType.Exp,
                                scale=EXP_SCALE)
                        es.append((kc, e))
                    while es:
                        drain_one()
                    rbs = rbsp.tile([P, TC], f32, tag="rbs",
                                    name=f"rbs_{t}_{head}")
                    nc.vector.reciprocal_approx_fast(rbs[:], d_rep[:])
                    nc.vector.tensor_tensor(
                        attnT[:, head, :], pv[:], rbs[:],
                        mybir.AluOpType.mult)

                # ---- phase 5: out chunk = attn(t) @ wo_shard ----
                for oc in range(NOC):
                    for tcn in range(TC // P):
                        o_ps = opsp.tile([P, TC], f32, tag="o",
                                         name=f"o_{t}_{oc}_{tcn}")
                        for hc in range(QH):
                            nc.tensor.matmul(
                                o_ps[:],
                                attnT[:, hc, tcn * P:(tcn + 1) * P],
                                wo_r[:, hc, oc * TC:(oc + 1) * TC],
                                start=(hc == 0), stop=(hc == QH - 1))
                        ob = obp.tile([P, TC], bf16, tag="ob",
                                      name=f"ob_{t}_{oc}_{tcn}")
                        nc.scalar.copy(ob[:], o_ps[:])
                        nc.gpsimd.dma_start(
                            part[t * TC + tcn * P:t * TC + (tcn + 1) * P,
                                 oc * TC:(oc + 1) * TC], ob[:])

    nc.compile()
    return nc


def _rope_tables(positions):
    pos = positions.astype(np.float64)
    inv_freq = 1.0 / (BASE ** (np.arange(HALF, dtype=np.float64) / HALF))
    freqs = pos[:, None] * inv_freq[None, :]          # [T, 64]
    cos = np.cos(freqs)
    sin = np.sin(freqs)
    cosT = np.concatenate([cos, cos], axis=1).T       # [128, T]
    sinT = np.concatenate([-sin, sin], axis=1).T      # sign folded
    return cosT, sinT


def kernel(positions, hidden_states, wqkv, wo):
    global _COMPILED
    if _COMPILED is None:
        _COMPILED = _build()
    nc = _COMPILED

    s = 128.0 ** -0.5                                 # per-side score scale
    cosT, sinT = _rope_tables(positions)
    cosq = np.ascontiguousarray(cosT * s).astype(ml_dtypes.bfloat16)
    sinq = np.ascontiguousarray(sinT * s).astype(ml_dtypes.bfloat16)

    hidT = np.ascontiguousarray(hidden_states.T).astype(ml_dtypes.bfloat16)

    # causal staircase mask, ST layout [k, q]: one [128,128] tile serves
    # every diagonal block
    kl = np.arange(P)[:, None]
    ql = np.arange(P)[None, :]
    masks = np.where(kl <= ql, 0.0, NEG).astype(ml_dtypes.bfloat16)

    ident = np.eye(P, dtype=np.float32).astype(ml_dtypes.bfloat16)
    onesm = np.ones((P, P), dtype=np.float32).astype(ml_dtypes.bfloat16)
    rperm_np = np.zeros((P, P), dtype=np.float32)
    for m in range(P):
        rperm_np[(m + HALF) % P, m] = 1.0             # out[m]=x[(m+64)%128]
    rperm_np = rperm_np.astype(ml_dtypes.bfloat16)

    in_maps = []
    for r in range(NCORES):
        qc = slice(r * QCOLS, (r + 1) * QCOLS)
        kc = slice(NH * HD + r * HD, NH * HD + (r + 1) * HD)
        vc = slice((NH + NKV) * HD + r * HD, (NH + NKV) * HD + (r + 1) * HD)
        wqkv_s = np.ascontiguousarray(
            np.concatenate([wqkv[:, qc], wqkv[:, kc], wqkv[:, vc]],
                           axis=1)).astype(ml_dtypes.bfloat16)
        wo_s = np.ascontiguousarray(wo[qc, :]).astype(ml_dtypes.bfloat16)
        in_maps.append({
            "hidT": hidT, "wqkv_s": wqkv_s, "wo_s": wo_s,
            "cosq": cosq, "sinq": sinq, "masks": masks,
            "ident": ident, "onesm": onesm, "rperm": rperm_np,
        })

    global _LAST_IN_MAPS
    _LAST_IN_MAPS = in_maps
    res = run_bass_kernel_spmd(nc, in_maps, list(range(NCORES)))
    out = res.results[0]["part"].astype(np.float64)
    for r in range(1, NCORES):
        out += res.results[r]["part"].astype(np.float64)
    return out.astype(np.float32)


# revision 30
# speedup vs baseline: 1.7681x; 1.0351x over previous
"""InternLM3 self-attention (prefill, GQA, RoPE) on 8 Trainium2 cores.

Tensor-parallel over heads: core r owns q heads 4r..4r+3 and kv head r
(wqkv column shards, wo row shards).  Each core computes its partial
output projection; the 8 partials are summed on the host.

v3 design:
  - all matmuls bf16 (Fast Weight Load on; fp32r would serialize
    LDWEIGHTS at ~125 ns/matmul).
  - one software pipeline over the 4 token chunks: proj(t) -> rope(t)
    -> attention(g=t) -> out-proj(t); per-tile semaphores overlap the
    phases across engines.
  - softmax denominator accumulated on PE with an all-ones [128,128]
    stationary (result replicated across partitions), so 1/d is a single
    reciprocal_approx_fast on [128,512] and the normalizing multiply
    needs no broadcast at all.
  - scores/pv/denominator matmuls run 2 chunks behind the scores stream
    so PSUM bank recycling at head boundaries never stalls the PE.
  - RoPE rotate-half via SBUF->SBUF DMA partition shift; k head is
    roped first so attention never waits on it.
  - q and k both carry 128^-0.5 in the rope tables; exp() applies the
    compensating sqrt(128) via its free affine scale.
  - batched weight DMAs (descriptors fan out across all 16 queues);
    wo load is emitted late so it never delays the first projection.
"""

import numpy as np
import ml_dtypes

import concourse.bass as bass
import concourse.bacc as bacc
import concourse.mybir as mybir
import concourse.tile as tile
from concourse.bass_utils import run_bass_kernel_spmd

T = 2048
H = 4096
NH = 32
NKV = 8
HD = 128
HALF = HD // 2
BASE = 1000000.0
NCORES = 8
QH = NH // NCORES            # 4 q heads per core
QCOLS = QH * HD              # 512
SH_COLS = QCOLS + 2 * HD     # 768 wqkv cols per core
NEG = -1e30

P = 128
TC = 512                     # token chunk
NT = T // TC                 # 4
NHC = H // P                 # 32 contraction chunks
NQC = SH_COLS // P           # 6 qkv col chunks (0-3 q heads, 4 k, 5 v)
NOC = H // TC                # 8 output col chunks
LAG = 2                      # chunks the pv/denominator stream trails by

f32 = mybir.dt.float32
bf16 = mybir.dt.bfloat16

EXP_SCALE = float(np.sqrt(128.0))   # tables carry 128^-0.5 on q and k

_COMPILED = None


def _build():
    nc = bacc.Bacc("TRN2", target_bir_lowering=False, debug=False,
                   num_devices=NCORES)

    hidT = nc.dram_tensor("hidT", [H, T], bf16, kind="ExternalInput").ap()
    wqkv_s = nc.dram_tensor("wqkv_s", [H, SH_COLS], bf16,
                            kind="ExternalInput").ap()
    wo_s = nc.dram_tensor("wo_s", [QCOLS, H], bf16,
                          kind="ExternalInput").ap()
    cosq = nc.dram_tensor("cosq", [P, T], bf16, kind="ExternalInput").ap()
    sinq = nc.dram_tensor("sinq", [P, T], bf16, kind="ExternalInput").ap()
    masks = nc.dram_tensor("masks", [P, P], bf16,
                           kind="ExternalInput").ap()
    ident = nc.dram_tensor("ident", [P, P], bf16, kind="ExternalInput").ap()
    onesm = nc.dram_tensor("onesm", [P, P], bf16, kind="ExternalInput").ap()
    rperm = nc.dram_tensor("rperm", [P, P], bf16, kind="ExternalInput").ap()
    part = nc.dram_tensor("part", [T, H], bf16, kind="ExternalOutput").ap()

    with tile.TileContext(nc) as tc:
        with tc.tile_pool(name="keep", bufs=1) as keep, \
             tc.tile_pool(name="hid", bufs=10) as hidp, \
             tc.tile_pool(name="attn", bufs=2) as attp, \
             tc.tile_pool(name="rot", bufs=2) as rotp, \
             tc.tile_pool(name="e", bufs=6) as ep, \
             tc.tile_pool(name="rbs", bufs=2) as rbsp, \
             tc.tile_pool(name="ob", bufs=6) as obp, \
             tc.tile_pool(name="ps", bufs=4, space="PSUM") as psp, \
             tc.tile_pool(name="acc", bufs=1, space="PSUM") as accp, \
             tc.tile_pool(name="ops", bufs=2, space="PSUM") as opsp:

            # ---- long-lived SBUF ----
            wq = keep.tile([P, NHC, SH_COLS], bf16)        # 48 KB/part
            wo_r = keep.tile([P, QH, H], bf16)             # 32 KB/part
            qkvT = keep.tile([P, NQC, T], bf16)            # 24 KB/part
            ct = keep.tile([P, T], bf16, tag="cosq_t")     # 4 KB
            st_t = keep.tile([P, T], bf16, tag="sinq_t")   # 4 KB
            mt = keep.tile([P, P], bf16, tag="masks_t")    # staircase
            vnat = keep.tile([P, T // P, P], bf16, tag="vnat")  # 4 KB
            idt = keep.tile([P, P], bf16, tag="ident_t")
            o_m = keep.tile([P, P], bf16, tag="onesm_t")
            rp = keep.tile([P, P], bf16, tag="rperm_t")

            # hid(0) + wq interleaved in first-needed order: the DMA queues
            # are FIFO, so issue order decides who gets early bandwidth.
            QTR = NHC // 8
            hid_t0 = [hidp.tile([P, QTR, TC], bf16, tag="hid",
                                name=f"hid_0_{qi}") for qi in range(8)]

            def hid_dma(hq, qi, t):
                nc.sync.dma_start(
                    hq[:],
                    hidT[qi * QTR * P:(qi + 1) * QTR * P,
                         t * TC:(t + 1) * TC].rearrange(
                        "(h p) n -> p h n", p=P))

            def wq_dma(c, half):
                hh = half * (NHC // 2)
                nc.sync.dma_start(
                    wq[:, hh:hh + NHC // 2, c * P:(c + 1) * P],
                    wqkv_s[hh * P:hh * P + NHC // 2 * P,
                           c * P:(c + 1) * P].rearrange(
                        "(h p) c -> p h c", p=P))

            def wq_dma_q(c, q8):
                hh = q8 * (NHC // 4)
                nc.sync.dma_start(
                    wq[:, hh:hh + NHC // 4, c * P:(c + 1) * P],
                    wqkv_s[hh * P:hh * P + NHC // 4 * P,
                           c * P:(c + 1) * P].rearrange(
                        "(h p) c -> p h c", p=P))

            hid_dma(hid_t0[0], 0, 0)
            wq_dma_q(4, 0)
            hid_dma(hid_t0[1], 1, 0)
            wq_dma_q(4, 1)
            hid_dma(hid_t0[2], 2, 0)
            wq_dma_q(4, 2)
            hid_dma(hid_t0[3], 3, 0)
            wq_dma_q(4, 3)
            for qi in range(4, 8):
                hid_dma(hid_t0[qi], qi, 0)
            for c in (5, 0, 1, 2, 3):
                wq_dma(c, 0)
                wq_dma(c, 1)
            nc.scalar.dma_start(ct[:], cosq[:])
            nc.scalar.dma_start(st_t[:], sinq[:])
            nc.scalar.dma_start(mt[:], masks[:])
            nc.scalar.dma_start(idt[:], ident[:])
            nc.scalar.dma_start(o_m[:], onesm[:])
            nc.scalar.dma_start(rp[:], rperm[:])

            for t in range(NT):
                tsl = slice(t * TC, (t + 1) * TC)

                if t == 0:
                    hid_q = hid_t0
                else:
                    hid_q = []
                    for qi in range(8):
                        hq = hidp.tile([P, QTR, TC], bf16, tag="hid",
                                       name=f"hid_{t}_{qi}")
                        hid_dma(hq, qi, t)
                        hid_q.append(hq)

                # ---- phase 1: qkv^T chunk = wqkv^T @ hidden^T ----
                # k and v first so rope(k) / v-transpose overlap the q cols.
                def proj_col(c):
                    qps = psp.tile([P, TC], f32, tag="ps",
                                   name=f"qps_{t}_{c}")
                    for h in range(NHC):
                        nc.tensor.matmul(
                            qps[:], wq[:, h, c * P:(c + 1) * P],
                            hid_q[h // QTR][:, h % QTR, :],
                            start=(h == 0), stop=(h == NHC - 1))
                    nc.scalar.copy(qkvT[:, c, tsl], qps[:])

                def rope_col(idx):
                    x = qkvT[:, idx, tsl]
                    rot_ps = psp.tile([P, TC], f32, tag="ps",
                                      name=f"rotp_{t}_{idx}")
                    nc.tensor.matmul(rot_ps[:], rp[:], x,
                                     start=True, stop=True)
                    rot = rotp.tile([P, TC], bf16, tag="rot",
                                    name=f"rot_{t}_{idx}")
                    nc.vector.tensor_tensor(
                        rot[:], rot_ps[:], st_t[:, tsl],
                        mybir.AluOpType.mult)
                    nc.vector.tensor_tensor(
                        x, x, ct[:, tsl], mybir.AluOpType.mult)
                    nc.vector.tensor_tensor(
                        x, x, rot[:], mybir.AluOpType.add)

                # rope/vt are emitted one c-group late so the PSUM->SBUF
                # copy they read never stalls the in-order PE stream.
                proj_col(4)                      # k
                proj_col(5)                      # v
                rope_col(QH)                     # k rope (c4 copy now done)
                proj_col(0)
                for j in range(TC // P):         # v transpose
                    kc = t * (TC // P) + j
                    tp = psp.tile([P, TC], bf16, tag="ps", name=f"vt_{kc}")
                    nc.tensor.transpose(
                        tp[:, 0:P], qkvT[:, 5, kc * P:(kc + 1) * P], idt[:])
                    nc.scalar.copy(vnat[:, kc, :], tp[:, 0:P])
                proj_col(1)
                rope_col(0)
                proj_col(2)
                rope_col(1)
                proj_col(3)
                rope_col(2)

                if t == 0:
                    # wo load, deferred so it never races the hot path
                    for hc in range(QH):
                        nc.sync.dma_start(wo_r[:, hc, :],
                                          wo_s[hc * P:(hc + 1) * P, :])

                # ---- phase 4: causal attention, q group g == t ----
                attnT = attp.tile([P, QH, TC], bf16, tag="attnT",
                                  name=f"attnT_{t}")
                kmax = 4 * (t + 1)
                for head in range(QH):
                    if head == 1:
                        rope_col(3)              # q3 rope hidden under head 0
                    d_rep = accp.tile([P, TC], f32, tag="d",
                                      name=f"d_{t}_{head}")
                    pv = accp.tile([P, TC], f32, tag="pv",
                                   name=f"pv_{t}_{head}")
                    es = []

                    def drain_one():
                        pkc, pe, plo = es.pop(0)
                        nc.tensor.matmul(d_rep[:, plo:], o_m[:],
                                         pe[:, plo:],
                                         start=(pkc == 0),
                                         stop=(pkc == kmax - 1))
                        nc.tensor.matmul(pv[:, plo:], vnat[:, pkc, :],
                                         pe[:, plo:],
                                         start=(pkc == 0),
                                         stop=(pkc == kmax - 1))

                    for kc in range(kmax):
                        j = kc - 4 * t
                        lo = max(j, 0) * P   # cols < lo are fully masked
                        st_ps = psp.tile([P, TC], f32, tag="ps",
                                         name=f"st_{t}_{head}_{kc}")
                        nc.tensor.matmul(
                            st_ps[:, lo:],
                            qkvT[:, QH, kc * P:(kc + 1) * P],
                            qkvT[:, head, t * TC + lo:(t + 1) * TC],
                            start=True, stop=True)
                        if len(es) >= LAG:
                            drain_one()
                        e = ep.tile([P, TC], bf16, tag="E",
                                    name=f"e_{t}_{head}_{kc}")
                        if j >= 0:
                            # staircase lives in cols [128j, 128j+128)
                            nc.vector.tensor_tensor(
                                st_ps[:, lo:lo + P],
                                st_ps[:, lo:lo + P], mt[:],
                                mybir.AluOpType.add)
                        nc.scalar.activation(
                            e[:, lo:], st_ps[:, lo:],
                            mybir.ActivationFunctionType.Exp,
                            scale=EXP_SCALE)
                        es.append((kc, e, lo))
                    while es:
                        drain_one()
                    rbs = rbsp.tile([P, TC], f32, tag="rbs",
                                    name=f"rbs_{t}_{head}")
                    nc.vector.reciprocal_approx_fast(rbs[:], d_rep[:])
                    nc.vector.tensor_tensor(
                        attnT[:, head, :], pv[:], rbs[:],
                        mybir.AluOpType.mult)

                # ---- phase 5: out chunk = attn(t) @ wo_shard ----
                for oc in range(NOC):
                    for tcn in range(TC // P):
                        o_ps = opsp.tile([P, TC], f32, tag="o",
                                         name=f"o_{t}_{oc}_{tcn}")
                        for hc in range(QH):
                            nc.tensor.matmul(
                                o_ps[:],
                                attnT[:, hc, tcn * P:(tcn + 1) * P],
                                wo_r[:, hc, oc * TC:(oc + 1) * TC],
                                start=(hc == 0), stop=(hc == QH - 1))
                        ob = obp.tile([P, TC], bf16, tag="ob",
                                      name=f"ob_{t}_{oc}_{tcn}")
                        nc.scalar.copy(ob[:], o_ps[:])
                        nc.gpsimd.dma_start(
                            part[t * TC + tcn * P:t * TC + (tcn + 1) * P,
                                 oc * TC:(oc + 1) * TC], ob[:])

    nc.compile()
    return nc


def _rope_tables(positions):
    pos = positions.astype(np.float64)
    inv_freq = 1.0 / (BASE ** (np.arange(HALF, dtype=np.float64) / HALF))
    freqs = pos[:, None] * inv_freq[None, :]          # [T, 64]
    cos = np.cos(freqs)
    sin = np.sin(freqs)
    cosT = np.concatenate([cos, cos], axis=1).T       # [128, T]
    sinT = np.concatenate([-sin, sin], axis=1).T      # sign folded
    return cosT, sinT


def kernel(positions, hidden_states, wqkv, wo):
    global _COMPILED
    if _COMPILED is None:
        _COMPILED = _build()
    nc = _COMPILED

    s = 128.0 ** -0.5                                 # per-side score scale
    cosT, sinT = _rope_tables(positions)
    cosq = np.ascontiguousarray(cosT * s).astype(ml_dtypes.bfloat16)
    sinq = np.ascontiguousarray(sinT * s).astype(ml_dtypes.bfloat16)

    hidT = np.ascontiguousarray(hidden_states.T).astype(ml_dtypes.bfloat16)

    # causal staircase mask, ST layout [k, q]: one [128,128] tile serves
    # every diagonal block
    kl = np.arange(P)[:, None]
    ql = np.arange(P)[None, :]
    masks = np.where(kl <= ql, 0.0, NEG).astype(ml_dtypes.bfloat16)

    ident = np.eye(P, dtype=np.float32).astype(ml_dtypes.bfloat16)
    onesm = np.ones((P, P), dtype=np.float32).astype(ml_dtypes.bfloat16)
    rperm_np = np.zeros((P, P), dtype=np.float32)
    for m in range(P):
        rperm_np[(m + HALF) % P, m] = 1.0             # out[m]=x[(m+64)%128]
    rperm_np = rperm_np.astype(ml_dtypes.bfloat16)

    in_maps = []
    for r in range(NCORES):
        qc = slice(r * QCOLS, (r + 1) * QCOLS)
        kc = slice(NH * HD + r * HD, NH * HD + (r + 1) * HD)
        vc = slice((NH + NKV) * HD + r * HD, (NH + NKV) * HD + (r + 1) * HD)
        wqkv_s = np.ascontiguousarray(
            np.concatenate([wqkv[:, qc], wqkv[:, kc], wqkv[:, vc]],
                           axis=1)).astype(ml_dtypes.bfloat16)
        wo_s = np.ascontiguousarray(wo[qc, :]).astype(ml_dtypes.bfloat16)
        in_maps.append({
            "hidT": hidT, "wqkv_s": wqkv_s, "wo_s": wo_s,
            "cosq": cosq, "sinq": sinq, "masks": masks,
            "ident": ident, "onesm": onesm, "rperm": rperm_np,
        })

    global _LAST_IN_MAPS
    _LAST_IN_MAPS = in_maps
    res = run_bass_kernel_spmd(nc, in_maps, list(range(NCORES)))
    out = res.results[0]["part"].astype(np.float64)
    for r in range(1, NCORES):
        out += res.results[r]["part"].astype(np.float64)
    return out.astype(np.float32)


# revision 31
# speedup vs baseline: 1.8402x; 1.0407x over previous
"""InternLM3 self-attention (prefill, GQA, RoPE) on 8 Trainium2 cores.

Tensor-parallel over heads: core r owns q heads 4r..4r+3 and kv head r
(wqkv column shards, wo row shards).  Each core computes its partial
output projection; the 8 partials are summed on the host.

v3 design:
  - all matmuls bf16 (Fast Weight Load on; fp32r would serialize
    LDWEIGHTS at ~125 ns/matmul).
  - one software pipeline over the 4 token chunks: proj(t) -> rope(t)
    -> attention(g=t) -> out-proj(t); per-tile semaphores overlap the
    phases across engines.
  - softmax denominator accumulated on PE with an all-ones [128,128]
    stationary (result replicated across partitions), so 1/d is a single
    reciprocal_approx_fast on [128,512] and the normalizing multiply
    needs no broadcast at all.
  - scores/pv/denominator matmuls run 2 chunks behind the scores stream
    so PSUM bank recycling at head boundaries never stalls the PE.
  - RoPE rotate-half via SBUF->SBUF DMA partition shift; k head is
    roped first so attention never waits on it.
  - q and k both carry 128^-0.5 in the rope tables; exp() applies the
    compensating sqrt(128) via its free affine scale.
  - batched weight DMAs (descriptors fan out across all 16 queues);
    wo load is emitted late so it never delays the first projection.
"""

import numpy as np
import ml_dtypes

import concourse.bass as bass
import concourse.bacc as bacc
import concourse.mybir as mybir
import concourse.tile as tile
from concourse.bass_utils import run_bass_kernel_spmd

T = 2048
H = 4096
NH = 32
NKV = 8
HD = 128
HALF = HD // 2
BASE = 1000000.0
NCORES = 8
QH = NH // NCORES            # 4 q heads per core
QCOLS = QH * HD              # 512
SH_COLS = QCOLS + 2 * HD     # 768 wqkv cols per core
NEG = -1e30

P = 128
TC = 512                     # token chunk
NT = T // TC                 # 4
NHC = H // P                 # 32 contraction chunks
NQC = SH_COLS // P           # 6 qkv col chunks (0-3 q heads, 4 k, 5 v)
NOC = H // TC                # 8 output col chunks
LAG = 3                      # chunks the pv/denominator stream trails by

f32 = mybir.dt.float32
bf16 = mybir.dt.bfloat16

EXP_SCALE = float(np.sqrt(128.0))   # tables carry 128^-0.5 on q and k

_COMPILED = None


def _build():
    nc = bacc.Bacc("TRN2", target_bir_lowering=False, debug=False,
                   num_devices=NCORES)

    hidT = nc.dram_tensor("hidT", [H, T], bf16, kind="ExternalInput").ap()
    wqkv_s = nc.dram_tensor("wqkv_s", [H, SH_COLS], bf16,
                            kind="ExternalInput").ap()
    wo_s = nc.dram_tensor("wo_s", [QCOLS, H], bf16,
                          kind="ExternalInput").ap()
    cosq = nc.dram_tensor("cosq", [P, T], bf16, kind="ExternalInput").ap()
    sinq = nc.dram_tensor("sinq", [P, T], bf16, kind="ExternalInput").ap()
    masks = nc.dram_tensor("masks", [P, P], bf16,
                           kind="ExternalInput").ap()
    ident = nc.dram_tensor("ident", [P, P], bf16, kind="ExternalInput").ap()
    onesm = nc.dram_tensor("onesm", [P, P], bf16, kind="ExternalInput").ap()
    rperm = nc.dram_tensor("rperm", [P, P], bf16, kind="ExternalInput").ap()
    part = nc.dram_tensor("part", [T, H], bf16, kind="ExternalOutput").ap()

    with tile.TileContext(nc) as tc:
        with tc.tile_pool(name="keep", bufs=1) as keep, \
             tc.tile_pool(name="hid", bufs=10) as hidp, \
             tc.tile_pool(name="attn", bufs=2) as attp, \
             tc.tile_pool(name="rot", bufs=2) as rotp, \
             tc.tile_pool(name="e", bufs=7) as ep, \
             tc.tile_pool(name="rbs", bufs=2) as rbsp, \
             tc.tile_pool(name="ob", bufs=6) as obp, \
             tc.tile_pool(name="ps", bufs=6, space="PSUM") as psp, \
             tc.tile_pool(name="acc", bufs=1, space="PSUM") as accp:

            # ---- long-lived SBUF ----
            wq = keep.tile([P, NHC, SH_COLS], bf16)        # 48 KB/part
            wo_r = keep.tile([P, QH, H], bf16)             # 32 KB/part
            qkvT = keep.tile([P, NQC, T], bf16)            # 24 KB/part
            ct = keep.tile([P, T], bf16, tag="cosq_t")     # 4 KB
            st_t = keep.tile([P, T], bf16, tag="sinq_t")   # 4 KB
            mt = keep.tile([P, P], bf16, tag="masks_t")    # staircase
            vnat = keep.tile([P, T // P, P], bf16, tag="vnat")  # 4 KB
            idt = keep.tile([P, P], bf16, tag="ident_t")
            o_m = keep.tile([P, P], bf16, tag="onesm_t")
            rp = keep.tile([P, P], bf16, tag="rperm_t")

            # hid(0) + wq interleaved in first-needed order: the DMA queues
            # are FIFO, so issue order decides who gets early bandwidth.
            QTR = NHC // 8
            hid_t0 = [hidp.tile([P, QTR, TC], bf16, tag="hid",
                                name=f"hid_0_{qi}") for qi in range(8)]

            def hid_dma(hq, qi, t):
                nc.sync.dma_start(
                    hq[:],
                    hidT[qi * QTR * P:(qi + 1) * QTR * P,
                         t * TC:(t + 1) * TC].rearrange(
                        "(h p) n -> p h n", p=P))

            def wq_dma(c, half):
                hh = half * (NHC // 2)
                nc.sync.dma_start(
                    wq[:, hh:hh + NHC // 2, c * P:(c + 1) * P],
                    wqkv_s[hh * P:hh * P + NHC // 2 * P,
                           c * P:(c + 1) * P].rearrange(
                        "(h p) c -> p h c", p=P))

            def wq_dma_q(c, q8):
                hh = q8 * (NHC // 4)
                nc.sync.dma_start(
                    wq[:, hh:hh + NHC // 4, c * P:(c + 1) * P],
                    wqkv_s[hh * P:hh * P + NHC // 4 * P,
                           c * P:(c + 1) * P].rearrange(
                        "(h p) c -> p h c", p=P))

            hid_dma(hid_t0[0], 0, 0)
            wq_dma_q(4, 0)
            hid_dma(hid_t0[1], 1, 0)
            wq_dma_q(4, 1)
            hid_dma(hid_t0[2], 2, 0)
            wq_dma_q(4, 2)
            hid_dma(hid_t0[3], 3, 0)
            wq_dma_q(4, 3)
            for qi in range(4, 8):
                hid_dma(hid_t0[qi], qi, 0)
            for c in (5, 0, 1, 2, 3):
                wq_dma(c, 0)
                wq_dma(c, 1)
            nc.scalar.dma_start(ct[:], cosq[:])
            nc.scalar.dma_start(st_t[:], sinq[:])
            nc.scalar.dma_start(mt[:], masks[:])
            nc.scalar.dma_start(idt[:], ident[:])
            nc.scalar.dma_start(o_m[:], onesm[:])
            nc.scalar.dma_start(rp[:], rperm[:])

            for t in range(NT):
                tsl = slice(t * TC, (t + 1) * TC)

                if t == 0:
                    hid_q = hid_t0
                else:
                    hid_q = []
                    for qi in range(8):
                        hq = hidp.tile([P, QTR, TC], bf16, tag="hid",
                                       name=f"hid_{t}_{qi}")
                        hid_dma(hq, qi, t)
                        hid_q.append(hq)

                # ---- phase 1: qkv^T chunk = wqkv^T @ hidden^T ----
                # k and v first so rope(k) / v-transpose overlap the q cols.
                def proj_col(c):
                    qps = psp.tile([P, TC], f32, tag="ps",
                                   name=f"qps_{t}_{c}")
                    for h in range(NHC):
                        nc.tensor.matmul(
                            qps[:], wq[:, h, c * P:(c + 1) * P],
                            hid_q[h // QTR][:, h % QTR, :],
                            start=(h == 0), stop=(h == NHC - 1))
                    nc.scalar.copy(qkvT[:, c, tsl], qps[:])

                def rope_col(idx):
                    x = qkvT[:, idx, tsl]
                    rot_ps = psp.tile([P, TC], f32, tag="ps",
                                      name=f"rotp_{t}_{idx}")
                    nc.tensor.matmul(rot_ps[:], rp[:], x,
                                     start=True, stop=True)
                    rot = rotp.tile([P, TC], bf16, tag="rot",
                                    name=f"rot_{t}_{idx}")
                    nc.vector.tensor_tensor(
                        rot[:], rot_ps[:], st_t[:, tsl],
                        mybir.AluOpType.mult)
                    nc.vector.tensor_tensor(
                        x, x, ct[:, tsl], mybir.AluOpType.mult)
                    nc.vector.tensor_tensor(
                        x, x, rot[:], mybir.AluOpType.add)

                # rope/vt are emitted one c-group late so the PSUM->SBUF
                # copy they read never stalls the in-order PE stream.
                proj_col(4)                      # k
                proj_col(5)                      # v
                rope_col(QH)                     # k rope (c4 copy now done)
                proj_col(0)
                for j in range(TC // P):         # v transpose
                    kc = t * (TC // P) + j
                    tp = psp.tile([P, TC], bf16, tag="ps", name=f"vt_{kc}")
                    nc.tensor.transpose(
                        tp[:, 0:P], qkvT[:, 5, kc * P:(kc + 1) * P], idt[:])
                    nc.scalar.copy(vnat[:, kc, :], tp[:, 0:P])
                proj_col(1)
                rope_col(0)
                proj_col(2)
                rope_col(1)
                proj_col(3)
                rope_col(2)

                if t == 0:
                    # wo load, deferred so it never races the hot path
                    for hc in range(QH):
                        nc.sync.dma_start(wo_r[:, hc, :],
                                          wo_s[hc * P:(hc + 1) * P, :])

                # ---- phase 4: causal attention, q group g == t ----
                attnT = attp.tile([P, QH, TC], bf16, tag="attnT",
                                  name=f"attnT_{t}")
                kmax = 4 * (t + 1)
                for head in range(QH):
                    if head == 1:
                        rope_col(3)              # q3 rope hidden under head 0
                    d_rep = accp.tile([P, TC], f32, tag="d",
                                      name=f"d_{t}_{head}")
                    pv = accp.tile([P, TC], f32, tag="pv",
                                   name=f"pv_{t}_{head}")
                    es = []

                    def drain_one():
                        pkc, pe, plo = es.pop(0)
                        nc.tensor.matmul(d_rep[:, plo:], o_m[:],
                                         pe[:, plo:],
                                         start=(pkc == 0),
                                         stop=(pkc == kmax - 1))
                        nc.tensor.matmul(pv[:, plo:], vnat[:, pkc, :],
                                         pe[:, plo:],
                                         start=(pkc == 0),
                                         stop=(pkc == kmax - 1))

                    for kc in range(kmax):
                        j = kc - 4 * t
                        lo = max(j, 0) * P   # cols < lo are fully masked
                        st_ps = psp.tile([P, TC], f32, tag="ps",
                                         name=f"st_{t}_{head}_{kc}")
                        nc.tensor.matmul(
                            st_ps[:, lo:],
                            qkvT[:, QH, kc * P:(kc + 1) * P],
                            qkvT[:, head, t * TC + lo:(t + 1) * TC],
                            start=True, stop=True)
                        if len(es) >= LAG:
                            drain_one()
                        e = ep.tile([P, TC], bf16, tag="E",
                                    name=f"e_{t}_{head}_{kc}")
                        if j >= 0:
                            # staircase lives in cols [128j, 128j+128)
                            nc.vector.tensor_tensor(
                                st_ps[:, lo:lo + P],
                                st_ps[:, lo:lo + P], mt[:],
                                mybir.AluOpType.add)
                        nc.scalar.activation(
                            e[:, lo:], st_ps[:, lo:],
                            mybir.ActivationFunctionType.Exp,
                            scale=EXP_SCALE)
                        es.append((kc, e, lo))
                    while es:
                        drain_one()
                    rbs = rbsp.tile([P, TC], f32, tag="rbs",
                                    name=f"rbs_{t}_{head}")
                    nc.vector.reciprocal_approx_fast(rbs[:], d_rep[:])
                    nc.vector.tensor_tensor(
                        attnT[:, head, :], pv[:], rbs[:],
                        mybir.AluOpType.mult)

                # ---- phase 5: out chunk = attn(t) @ wo_shard ----
                for oc in range(NOC):
                    for tcn in range(TC // P):
                        o_ps = psp.tile([P, TC], f32, tag="ps",
                                         name=f"o_{t}_{oc}_{tcn}")
                        for hc in range(QH):
                            nc.tensor.matmul(
                                o_ps[:],
                                attnT[:, hc, tcn * P:(tcn + 1) * P],
                                wo_r[:, hc, oc * TC:(oc + 1) * TC],
                                start=(hc == 0), stop=(hc == QH - 1))
                        ob = obp.tile([P, TC], bf16, tag="ob",
                                      name=f"ob_{t}_{oc}_{tcn}")
                        if (oc + tcn) % 2 == 0:
                            nc.scalar.copy(ob[:], o_ps[:])
                        else:
                            nc.vector.tensor_copy(ob[:], o_ps[:])
                        nc.gpsimd.dma_start(
                            part[t * TC + tcn * P:t * TC + (tcn + 1) * P,
                                 oc * TC:(oc + 1) * TC], ob[:])

    nc.compile()
    return nc


def _rope_tables(positions):
    pos = positions.astype(np.float64)
    inv_freq = 1.0 / (BASE ** (np.arange(HALF, dtype=np.float64) / HALF))
    freqs = pos[:, None] * inv_freq[None, :]          # [T, 64]
    cos = np.cos(freqs)
    sin = np.sin(freqs)
    cosT = np.concatenate([cos, cos], axis=1).T       # [128, T]
    sinT = np.concatenate([-sin, sin], axis=1).T      # sign folded
    return cosT, sinT


def kernel(positions, hidden_states, wqkv, wo):
    global _COMPILED
    if _COMPILED is None:
        _COMPILED = _build()
    nc = _COMPILED

    s = 128.0 ** -0.5                                 # per-side score scale
    cosT, sinT = _rope_tables(positions)
    cosq = np.ascontiguousarray(cosT * s).astype(ml_dtypes.bfloat16)
    sinq = np.ascontiguousarray(sinT * s).astype(ml_dtypes.bfloat16)

    hidT = np.ascontiguousarray(hidden_states.T).astype(ml_dtypes.bfloat16)

    # causal staircase mask, ST layout [k, q]: one [128,128] tile serves
    # every diagonal block
    kl = np.arange(P)[:, None]
    ql = np.arange(P)[None, :]
    masks = np.where(kl <= ql, 0.0, NEG).astype(ml_dtypes.bfloat16)

    ident = np.eye(P, dtype=np.float32).astype(ml_dtypes.bfloat16)
    onesm = np.ones((P, P), dtype=np.float32).astype(ml_dtypes.bfloat16)
    rperm_np = np.zeros((P, P), dtype=np.float32)
    for m in range(P):
        rperm_np[(m + HALF) % P, m] = 1.0             # out[m]=x[(m+64)%128]
    rperm_np = rperm_np.astype(ml_dtypes.bfloat16)

    in_maps = []
    for r in range(NCORES):
        qc = slice(r * QCOLS, (r + 1) * QCOLS)
        kc = slice(NH * HD + r * HD, NH * HD + (r + 1) * HD)
        vc = slice((NH + NKV) * HD + r * HD, (NH + NKV) * HD + (r + 1) * HD)
        wqkv_s = np.ascontiguousarray(
            np.concatenate([wqkv[:, qc], wqkv[:, kc], wqkv[:, vc]],
                           axis=1)).astype(ml_dtypes.bfloat16)
        wo_s = np.ascontiguousarray(wo[qc, :]).astype(ml_dtypes.bfloat16)
        in_maps.append({
            "hidT": hidT, "wqkv_s": wqkv_s, "wo_s": wo_s,
            "cosq": cosq, "sinq": sinq, "masks": masks,
            "ident": ident, "onesm": onesm, "rperm": rperm_np,
        })

    global _LAST_IN_MAPS
    _LAST_IN_MAPS = in_maps
    res = run_bass_kernel_spmd(nc, in_maps, list(range(NCORES)))
    out = res.results[0]["part"].astype(np.float64)
    for r in range(1, NCORES):
        out += res.results[r]["part"].astype(np.float64)
    return out.astype(np.float32)
